# revision 60
# baseline (speedup 1.0000x reference)
"""Trainium2 Bass kernel for nn_EnsembleNet3 (gnn_message_passing).

Self-contained: takes full inputs (as produced by setup_inputs()), shards the
B=32 graph dim over 8 NeuronCores (4 graphs/core), runs the graph stack fully
on-device, and computes the [B,1536] head sharded over output columns with
AllGathers between layers.

Per graph (N=512 nodes):
- kNN-100 for TAGConv: per-row threshold via count-secant iteration on
  Q[i,j] = 2*G[i,j] - n_j (same per-row order as -dist; Q row max is self),
  finished by an exact masked max-8 endgame; adjacency applied as a dense 0/1
  mask matmul on PE with host-folded hop weights:
  out = h@W~0 + (Mh)@W~1 + (M^2 h)@W~2,  M = mask incl self.
- EdgeConv k=3: top-3 indices via max8+max_index on Q; gather done as an
  exact one-hot selection matmul on PE: Sel_t[j,i] = (idx_l(i) == 128t+j)
  built by DVE compares from a PE-replicated index row, then
  gathered = g^T~t . Sel_t accumulated over tiles in PSUM. MLPs decomposed
  as a_i + g_j so only g is gathered; max-aggregation commutes with
  leaky_relu. leaky_relu computed as max(x, 0.01x) in one fused DVE op
  (with free mean-pool accumulation) or via ACT Lrelu with bias folding.
"""
import os
from contextlib import ExitStack

import numpy as np

import concourse.bass as bass
import concourse.bacc as bacc
import concourse.tile as tile
from concourse import mybir
from concourse._compat import with_exitstack

F32 = mybir.dt.float32
F32R = mybir.dt.float32r
F16 = mybir.dt.float16
U16 = mybir.dt.uint16
U32 = mybir.dt.uint32
U8 = mybir.dt.uint8
ALU = mybir.AluOpType
ACTF = mybir.ActivationFunctionType
AXX = mybir.AxisListType.X

B, N, F, W = 32, 512, 6, 128
NT = N // 128
GPC = 4
NCORES = 8
K101 = 101
SEL_ITERS = 8
SEL_TARGET = float(K101 + 4)
U_LO, U_HI = -16.0, 16.0
DIM2 = 1536
HSH = DIM2 // NCORES
LIN_D = 5
DVE_COLS = 7    # selection count passes: cols < DVE_COLS on DVE, rest on ACT


def _fold_tag(Wk, b):
    W0, W1, W2 = Wk[0], Wk[1], Wk[2]
    c1, c2 = 1.0 / 100.0, 1.0 / 10000.0
    return (
        (W0 - W1 * c1 + W2 * c2).astype(np.float32),
        (W1 * c1 - 2.0 * W2 * c2).astype(np.float32),
        (W2 * c2).astype(np.float32),
        b.astype(np.float32),
    )


def prep_host(inputs, core):
    inp = {k: np.asarray(v) for k, v in inputs.items()}
    x = inp['x'].astype(np.float32).reshape(B, N, F)
    xs = x[GPC * core:GPC * (core + 1)]
    f32 = np.float32

    # --- per-graph input pack [128, 1048]: xgT | xgR | xnm ---
    xt = xs.transpose(0, 2, 1)
    xpack = np.zeros((GPC, 128, 2 * N + 24), f32)
    xpack[:, 0:F, 0:N] = xt
    xpack[:, F:2 * F, 0:N] = 1.0
    xpack[:, 0:F, N:2 * N] = 2.0 * xt
    xpack[:, F:2 * F, N:2 * N] = -(xt * xt)
    xpack[:, 12, N:2 * N] = 1.0
    for t in range(NT):
        xpack[:, :, 2 * N + F * t:2 * N + F * (t + 1)] = xs[:, 128 * t:128 * (t + 1), :]

    # --- const pack [128, cols] ---
    cols = {}
    blobs = []
    off = 0

    def put(name, arr2d):
        nonlocal off
        a = np.asarray(arr2d, f32)
        pad = np.zeros((128, a.shape[1]), f32)
        pad[:a.shape[0]] = a
        cols[name] = (off, a.shape[1])
        blobs.append(pad)
        off += a.shape[1]

    put('eye', np.eye(128, dtype=f32))
    put('iota8', np.broadcast_to(np.arange(8, dtype=f32), (128, 8)))
    put('iotaP', np.arange(128, dtype=f32).reshape(128, 1))
    rsel = np.zeros((3, 384), f32)
    for l in range(3):
        rsel[l, 128 * l:128 * (l + 1)] = 1.0
    put('rsel3', rsel)
    for li, (Wk, bk) in enumerate(
            [(inp['tag1_W'], inp['tag1_b']),
             (inp['tag_W'][0], inp['tag_b'][0]),
             (inp['tag_W'][1], inp['tag_b'][1])]):
        w0, w1, w2, bb = _fold_tag(Wk, bk)
        put(f'tagw{li}', np.concatenate([w0, w1, w2], axis=1))
        put(f'tagb{li}', bb.reshape(128, 1))
    W1 = inp['p1_W1'].astype(f32)
    put('ec1_a', W1[:F] - W1[F:])
    put('ec1_g', W1[F:])
    put('ec1_b1', inp['p1_b1'].astype(f32).reshape(128, 1))
    put('ec1_w2', inp['p1_W2'].astype(f32))
    put('ec1_b2', inp['p1_b2'].astype(f32).reshape(128, 1))
    for f in range(2):
        Wf = inp['pf_W'][f].astype(f32)
        put(f'ec{f+2}_a', Wf[:W] - Wf[W:])
        put(f'ec{f+2}_g', Wf[W:])
        put(f'ec{f+2}_b', inp['pf_b'][f].astype(f32).reshape(128, 1))
    put('bn_scale', inp['bn_gamma'].astype(f32).reshape(12, 128).T)
    put('bn_shift', inp['bn_beta'].astype(f32).reshape(12, 128).T)
    put('outW', inp['out_W'].astype(f32).reshape(12, 128).T)
    put('outb', np.full((1, 1), float(inp['out_b'][0]), f32))
    sl = slice(HSH * core, HSH * (core + 1))
    put('linB', inp['lin_b'].astype(f32)[:, sl].reshape(1, LIN_D * HSH))
    lbc = np.zeros((128, 2 * LIN_D), f32)
    for li in range(LIN_D):
        lb = inp['lin_b'].astype(f32)[li, sl]
        lbc[0:128, 2 * li] = lb[0:128]
        lbc[0:64, 2 * li + 1] = lb[128:192]
    put('linBcol', lbc)
    wpack = np.concatenate(blobs, axis=1)
    assert wpack.shape[1] == WPACK_COLS, (wpack.shape, off)
    assert all(cols[k] == WOFF[k] for k in cols), "WOFF mismatch"

    linW = inp['lin_W'].astype(np.float16)
    d = {
        'xpack': np.ascontiguousarray(xpack),
        'wpack': np.ascontiguousarray(wpack),
        'linW': np.ascontiguousarray(linW[:, :, sl].reshape(LIN_D, 12, 128, HSH)),
    }
    return d


def _woff_table():
    off = 0
    tab = {}
    def put(name, w):
        nonlocal off
        tab[name] = (off, w)
        off += w
    put('eye', 128); put('iota8', 8); put('iotaP', 1); put('rsel3', 384)
    for li in range(3):
        put(f'tagw{li}', 384); put(f'tagb{li}', 1)
    put('ec1_a', 128); put('ec1_g', 128); put('ec1_b1', 1)
    put('ec1_w2', 128); put('ec1_b2', 1)
    for f in range(2):
        put(f'ec{f+2}_a', 128); put(f'ec{f+2}_g', 128); put(f'ec{f+2}_b', 1)
    put('bn_scale', 12); put('bn_shift', 12)
    put('outW', 12); put('outb', 1)
    put('linB', LIN_D * HSH); put('linBcol', 2 * LIN_D)
    return tab, off


WOFF, WPACK_COLS = _woff_table()


@with_exitstack
def core_program(ctx: ExitStack, tc: tile.TileContext, io: dict, skip_head=False):
    nc = tc.nc
    P = 128

    const = ctx.enter_context(tc.tile_pool(name="const", bufs=1))
    pq = ctx.enter_context(tc.tile_pool(name="pq", bufs=16))
    pq2 = ctx.enter_context(tc.tile_pool(name="pq2", bufs=1))
    pmask = ctx.enter_context(tc.tile_pool(name="pmask", bufs=8))
    pwork = ctx.enter_context(tc.tile_pool(name="pwork", bufs=1))
    psel = ctx.enter_context(tc.tile_pool(name="psel", bufs=8))
    phid = ctx.enter_context(tc.tile_pool(name="phid", bufs=2))
    phT = ctx.enter_context(tc.tile_pool(name="phT", bufs=4))
    pyT = ctx.enter_context(tc.tile_pool(name="pyT", bufs=3))
    phn = ctx.enter_context(tc.tile_pool(name="phn", bufs=8))
    pst = ctx.enter_context(tc.tile_pool(name="pst", bufs=1))
    phw = ctx.enter_context(tc.tile_pool(name="phw", bufs=1))
    psq = ctx.enter_context(tc.tile_pool(name="psq", bufs=4, space="PSUM"))
    pss = ctx.enter_context(tc.tile_pool(name="pss", bufs=2, space="PSUM"))

    def quad_ps(pp=P, nn=N):
        return psq.tile([pp, nn], F32, tag="quad", name="quad")

    def seq_ps(pp, nn):
        return pss.tile([pp, nn], F32, tag="seq", name="seq")

    def dma(dst, src):
        nc.sync.dma_start(dst, src)

    # All value matmuls are exact fp32: the head BatchNorm divides pooled
    # features by ~3e-3 (tiny batch variance), amplifying any value-path
    # rounding ~300x; f32r's 2.4e-4 rounding would blow the 2e-2 gate.
    # f32r is used ONLY where operands are exact small ints / 0-1 (rep).
    def mmf(out, lhsT, rhs, start, stop):
        nc.tensor.matmul(out, lhsT, rhs, start=start, stop=stop)

    def mmr(out, lhsT, rhs, start, stop):
        # f32r: 1 cyc/row vs fp32's 4. Only on paths that tolerate the
        # ~2.4e-4 operand rounding (gather payloads, 2nd-hop aggregates);
        # never on x/y/weights that feed rankings or the pools directly.
        nc.tensor.matmul(out, lhsT.bitcast(F32R), rhs.bitcast(F32R),
                         start=start, stop=stop)

    # ---- constants: one packed DMA ----
    # DMA'd with f32r dtype so the weights count as f32r-rounded producers
    # for the walrus verifier (bytes unchanged; PE truncates on read).
    wp = const.tile([P, WPACK_COLS], F32, tag="wpack", name="wpack")
    dma(wp[:], io['wpack'][:])

    def wslice(name, rows=128):
        o, w = WOFF[name]
        return wp[0:rows, o:o + w]

    eye = wslice('eye')
    iota8 = wslice('iota8')
    iotaP = wslice('iotaP')
    rsel3 = wslice('rsel3', 3)
    ones_col = const.tile([P, 1], F32)
    nc.any.memset(ones_col[:], 1.0)
    ones32 = const.tile([1, 32], F32, padded_shape=[128, 32])
    nc.any.memset(ones32[:], 1.0)
    allones = const.tile([P, P], F32)
    nc.any.memset(allones[:], 1.0)

    tagw, tagb = [], []
    for li in range(3):
        fin = F if li == 0 else W
        wt = wslice(f'tagw{li}', fin)
        tagw.append([wt[:, 128 * k:128 * (k + 1)] for k in range(3)])
        tagb.append(wslice(f'tagb{li}'))

    rs_o, rs_w = WOFF['rsel3']
    rsel3r_t = const.tile([P, rs_w], F32, tag="r_rsel3", name="r_rsel3")
    nc.vector.tensor_copy(rsel3r_t[:].bitcast(F32R), wp[0:P, rs_o:rs_o + rs_w])
    rsel3r = rsel3r_t[0:3, :]

    ec1_a = wslice('ec1_a', F)
    ec1_g = wslice('ec1_g', F)
    ec1_w2 = wslice('ec1_w2')
    ec1_b1 = wslice('ec1_b1')
    ec1_b2 = wslice('ec1_b2')
    ecf_a = [wslice('ec2_a'), wslice('ec3_a')]
    ecf_g = [wslice('ec2_g'), wslice('ec3_g')]
    ecf_b = [wslice('ec2_b'), wslice('ec3_b')]

    # ---- inputs per graph: one packed DMA each ----
    xgT, xgR, xnm = [], [], []
    for g in range(GPC):
        xp = pst.tile([P, 2 * N + 24], F32, tag=f"xpack{g}", name=f"xpack{g}")
        dma(xp[:], io['xpack'][g])
        xgT.append(xp[:, 0:N])
        xgR.append(xp[:, N:2 * N])
        xnm.append([xp[:, 2 * N + F * t:2 * N + F * (t + 1)] for t in range(NT)])

    # ---- prefetch head weights early (fp16): streams during graph phase ----
    wts = []
    for li in range(LIN_D):
        wt = phw.tile([P, 12 * HSH], F16, tag=f"linW{li}", name=f"linW{li}")
        dma(wt[:].rearrange("a (k b) -> a k b", k=12),
            io['linW'][li].rearrange("k a b -> a k b"))
        wts.append(wt)

    # ---- Q = 2G - n_row via augmented matmul (K=12), n_col from x_nm ----
    Q = [[None] * NT for _ in range(GPC)]
    for g in range(GPC):
        gps = [quad_ps() for _ in range(NT)]
        for t in range(NT):
            nc.tensor.matmul(gps[t][:], xgT[g][0:12, 128 * t:128 * (t + 1)],
                             xgR[g][0:12, 0:N], start=True, stop=True)
        for t in range(NT):
            qt = pq.tile([P, N], F32, tag="Q", name="Q")
            nc.scalar.copy(qt[:], gps[t][:])
            Q[g][t] = qt

    # ---- per-graph count-secant selection (pipelines across graphs) ----
    # DVE cols [0, SEL_DVE), ACT cols [SEL_DVE, NT) per graph per iteration.
    SEL_DVE = 2
    pjk = ctx.enter_context(tc.tile_pool(name="pjk", bufs=2))

    def sel_graph(g):
        """Exact 101st-largest threshold per Q row -> ustar_g [P, NT]."""
        def t4(nm, init=None):
            tt = pst.tile([P, NT], F32, tag=f"{nm}{g}", name=f"{nm}{g}")
            if init is not None:
                nc.any.memset(tt[:], init)
            return tt
        st_u = t4('su', U_HI + (U_LO - U_HI) * (SEL_TARGET / N))
        st_ul = t4('sul', U_HI)
        st_uh = t4('suh', U_LO)
        st_cl = t4('scl', 0.0)
        st_ch = t4('sch', float(N))
        cnt = t4('scn')
        tmp_a = t4('sta')
        tmp_b = t4('stb')
        tmp_m = pst.tile([P, NT], U8, tag=f"stm{g}", name=f"stm{g}")
        for it in range(SEL_ITERS):
            for t in range(NT):
                ucol = st_u[:, t:t + 1]
                ccol = cnt[:, t:t + 1]
                if t < SEL_DVE:
                    jd = pjk.tile([P, N], F32, tag="junkd", name="junkd")
                    nc.vector.tensor_scalar(
                        jd[:], Q[g][t][:], ucol, 0.0,
                        op0=ALU.is_ge, op1=ALU.add, accum_out=ccol)
                else:
                    ja = pjk.tile([P, N], F32, tag="junka", name="junka")
                    nc.scalar.activation(
                        ja[:], Q[g][t][:], ACTF.Sign,
                        bias=ucol, scale=-1.0, accum_out=ccol)
            # ACT cols report sum(sign): c = 256 - s/2
            nc.vector.tensor_scalar(
                cnt[:, SEL_DVE:NT], cnt[:, SEL_DVE:NT], -0.5, 256.0,
                op0=ALU.mult, op1=ALU.add)
            nc.vector.tensor_scalar(
                tmp_m[:], cnt[:], float(K101) - 0.5, 0.0, op0=ALU.is_ge)
            nc.vector.copy_predicated(st_uh[:], tmp_m[:], st_u[:])
            nc.vector.copy_predicated(st_ch[:], tmp_m[:], cnt[:])
            nc.vector.tensor_scalar(
                tmp_m[:], cnt[:], float(K101) - 0.5, 0.0, op0=ALU.is_lt)
            nc.vector.copy_predicated(st_ul[:], tmp_m[:], st_u[:])
            nc.vector.copy_predicated(st_cl[:], tmp_m[:], cnt[:])
            if it == SEL_ITERS - 1:
                break
            nc.vector.tensor_tensor(tmp_a[:], st_ch[:], st_cl[:],
                                    op=ALU.subtract)
            nc.vector.reciprocal(tmp_a[:], tmp_a[:])
            nc.vector.scalar_tensor_tensor(
                tmp_b[:], st_ch[:], -SEL_TARGET, tmp_a[:],
                op0=ALU.add, op1=ALU.mult)
            nc.vector.tensor_scalar(
                tmp_b[:], tmp_b[:], 0.05, 0.95, op0=ALU.max, op1=ALU.min)
            nc.vector.tensor_tensor(tmp_a[:], st_ul[:], st_uh[:],
                                    op=ALU.subtract)
            nc.vector.tensor_tensor(tmp_a[:], tmp_a[:], tmp_b[:], op=ALU.mult)
            nc.vector.tensor_tensor(st_u[:], st_uh[:], tmp_a[:], op=ALU.add)
        # endgame: exact 101st-largest via masked top-8
        ustar_g = t4('sus')
        pos_g = t4('sps')
        nc.vector.tensor_scalar(pos_g[:], st_ch[:], -float(K101), 0.0,
                                op0=ALU.add)
        for t in range(NT):
            zm = pwork.tile([P, N], F32, tag="zm", name="zm")
            nc.vector.tensor_scalar(
                zm[:], Q[g][t][:], st_uh[:, t:t + 1], -1e30,
                op0=ALU.is_lt, op1=ALU.mult)
            nc.vector.tensor_tensor(zm[:], zm[:], Q[g][t][:], op=ALU.subtract)
            m8 = pwork.tile([P, 8], F32, tag="m8e", name="m8e")
            nc.vector.max(m8[:], zm[:])
            msk8 = pwork.tile([P, 8], F32, tag="msk8", name="msk8")
            nc.vector.tensor_tensor(
                msk8[:], iota8,
                pos_g[:, t:t + 1].broadcast_to([P, 8]), op=ALU.is_equal)
            j8 = pwork.tile([P, 8], F32, tag="j8", name="j8")
            nc.vector.scalar_tensor_tensor(
                j8[:], m8[:], -1.0, msk8[:], op0=ALU.mult, op1=ALU.mult,
                accum_out=ustar_g[:, t:t + 1])
        return ustar_g

    lrelu_op = dict(op0=ALU.mult, op1=ALU.max)

    def lrelu_into(dst, src, accum=None):
        nc.vector.scalar_tensor_tensor(dst, src, 0.01, src, accum_out=accum,
                                       **lrelu_op)

    def topk3(Qt, lo=1):
        """top-3 neighbor indices per node -> ts3s [3, N] fp32.

        Takes ranks lo..lo+2. lo=1 skips rank 0 (self, when Q row max is
        self); lo=0 is used when the diagonal is already masked out.
        """
        ts3 = seq_ps(3, N)
        for t in range(NT):
            m8 = pwork.tile([P, 8], F32, tag="m8g", name="m8g")
            nc.vector.max(m8[:], Qt[t][:])
            i8 = pwork.tile([P, 8], U32, tag="i8g", name="i8g")
            nc.vector.max_index(i8[:], m8[:], Qt[t][:])
            i8f = pwork.tile([P, 8], F32, tag="i8f", name="i8f")
            nc.vector.tensor_copy(i8f[:], i8[:])
            nc.tensor.transpose(ts3[0:3, 128 * t:128 * (t + 1)],
                                i8f[:, lo:lo + 3], eye)
        ts3s = pwork.tile([3, N], F32, tag="ts3s", name="ts3s", padded_shape=[128, N])
        nc.scalar.copy(ts3s[:].bitcast(F32R), ts3[0:3, :])
        return ts3s

    def sel_gather(ts3s, l, lhsTs, ps, close=True):
        """ps[f, i] (+)= payload[f, idx_l(i)] via one-hot Sel matmuls.

        Sel_t[j, i] = (idx_l(i) == 128t + j), built by DVE compare from a
        PE-replicated index row; lhsTs[t] = node-major payload [j, f].
        """
        rep_ps = seq_ps(P, N)
        nc.tensor.matmul(rep_ps[:],
                         rsel3r[0:3, 128 * l:128 * (l + 1)].bitcast(F32R),
                         ts3s[0:3, 0:N].bitcast(F32R), start=True, stop=True)
        for t in range(NT):
            sel = psel.tile([P, N], F32, tag="sel", name="sel")
            nc.vector.tensor_scalar(
                sel[:], rep_ps[:], iotaP, float(128 * t),
                op0=ALU.subtract, op1=ALU.is_equal)
            mmf(ps[:], lhsTs[t][:], sel[:], start=(t == 0),
                stop=(close and t == NT - 1))

    zpack = [pst.tile([P, 12], F32, tag=f"zpack{g}", name=f"zpack{g}") for g in range(GPC)]
    dbg = globals().get('_DEBUG_IO')

    for g in range(GPC):
        ustar_g = sel_graph(g)
        # ---- maskT: M[i,j] = (Q[i,j] >= u*_i) on DVE, then PE transposes ----
        Mrows = []
        for t in range(NT):
            mrow = pwork.tile([P, N], F32, tag="Mrow", name="Mrow")
            nc.vector.tensor_scalar(
                mrow[:], Q[g][t][:], ustar_g[:, t:t + 1], 0.0,
                op0=ALU.is_ge)
            Mrows.append(mrow)
        maskT = []
        for jc in range(NT):
            mps = quad_ps()
            for it in range(NT):
                nc.tensor.transpose(mps[0:P, 128 * it:128 * (it + 1)],
                                    Mrows[it][:, 128 * jc:128 * (jc + 1)], eye)
            mt = pmask.tile([P, N], F32, tag="maskT", name="maskT")
            nc.scalar.copy(mt[:], mps[:])
            maskT.append(mt)

        # ---- TAG ----
        hT = xgT[g][0:F, 0:N]
        hnm = xnm[g]
        for li in range(3):
            fin = F if li == 0 else W
            if li == 0:
                # fin=6: node-major aggregation is cheap (N=6 matmuls)
                u1n_ps = [quad_ps(P, fin) for _ in range(NT)]
                for ic in range(NT):
                    for jc in range(NT):
                        mmf(u1n_ps[ic][0:P, 0:fin],
                            maskT[jc][:, 128 * ic:128 * (ic + 1)],
                            hnm[jc][:], start=(jc == 0), stop=(jc == NT - 1))
                u1n = []
                for ic in range(NT):
                    uu = phn.tile([P, fin], F32, tag="u1n", name="u1n")
                    nc.scalar.copy(uu[:], u1n_ps[ic][0:P, 0:fin])
                    u1n.append(uu)
                u1T_ps = seq_ps(fin, N)
                for ic in range(NT):
                    nc.tensor.transpose(u1T_ps[0:fin, 128 * ic:128 * (ic + 1)],
                                        u1n[ic][:], eye)
                u1T = pwork.tile([fin, N], F32, tag="u1T", name="u1T",
                                 padded_shape=[128, N])
                nc.scalar.copy(u1T[:], u1T_ps[0:fin, :])
            else:
                # fin=128: aggregate feature-major (4 wide matmuls), then
                # transpose to node-major for the second hop.
                u1T_ps = seq_ps(fin, N)
                for jc in range(NT):
                    mmf(u1T_ps[0:fin, :], hnm[jc][:], maskT[jc][:],
                        start=(jc == 0), stop=(jc == NT - 1))
                u1T = pwork.tile([fin, N], F32, tag="u1T", name="u1T",
                                 padded_shape=[128, N])
                nc.scalar.copy(u1T[:], u1T_ps[0:fin, :])
                u1n = []
                for ic in range(NT):
                    ups = quad_ps(P, P)
                    nc.tensor.transpose(ups[0:P, 0:P],
                                        u1T[:, 128 * ic:128 * (ic + 1)], eye)
                    uu = phn.tile([P, fin], F32, tag="u1n", name="u1n")
                    nc.scalar.copy(uu[:], ups[0:P, 0:P])
                    u1n.append(uu)
            u2T_ps = seq_ps(fin, N)
            for jc in range(NT):
                mmf(u2T_ps[0:fin, :], u1n[jc][:], maskT[jc][:],
                    start=(jc == 0), stop=(jc == NT - 1))
            u2T = pwork.tile([fin, N], F32, tag="u2T", name="u2T", padded_shape=[128, N])
            nc.scalar.copy(u2T[:], u2T_ps[0:fin, :])
            oT_ps = seq_ps(P, N)
            nc.tensor.matmul(oT_ps[:], tagw[li][0], hT[:], start=True, stop=False)
            nc.tensor.matmul(oT_ps[:], tagw[li][1], u1T[:], start=False, stop=False)
            nc.tensor.matmul(oT_ps[:], tagw[li][2], u2T[:], start=False, stop=True)
            sT = pwork.tile([P, N], F32, tag="sT", name="sT")
            nc.scalar.activation(sT[:], oT_ps[:], ACTF.Identity, bias=tagb[li])
            hT_new = phT.tile([P, N], F32, tag="hT", name="hT")
            lrelu_into(hT_new[:], sT[:],
                       accum=zpack[g][:, 2 * li:2 * li + 1])
            nc.vector.tensor_reduce(zpack[g][:, 2 * li + 1:2 * li + 2], hT_new[:],
                                    axis=AXX, op=ALU.max)
            if g == 0 and dbg is not None and 'dbgH' in dbg:
                dma(dbg['dbgH'][li], hT_new[:])
            hT = hT_new
            if li < 2:
                hnm = []
                for t in range(NT):
                    hps = quad_ps(P, P)
                    nc.tensor.transpose(hps[0:P, 0:P], hT[:, 128 * t:128 * (t + 1)],
                                        eye)
                    hh = phn.tile([P, P], F32, tag="hnm", name="hnm")
                    nc.scalar.copy(hh[:], hps[0:P, 0:P])
                    hnm.append(hh)

        # ---- EC1 ----
        g1T = []
        for t in range(NT):
            gps = quad_ps(P, P)
            nc.tensor.matmul(gps[0:P, 0:P], xgT[g][0:F, 128 * t:128 * (t + 1)],
                             ec1_g, start=True, stop=True)
            gt = phn.tile([P, P], F32, tag="gT", name="gT")
            nc.scalar.copy(gt[:], gps[0:P, 0:P])
            g1T.append(gt)
        ts1 = topk3(Q[g])
        mx = pwork.tile([P, N], F32, tag="mx", name="mx")
        for l in range(3):
            hid_ps = quad_ps()
            sel_gather(ts1, l, g1T, hid_ps, close=False)
            mmf(hid_ps[:], ec1_a, xgT[g][0:F, 0:N], start=False, stop=True)
            hid = phid.tile([P, N], F32, tag="hid", name="hid")
            nc.scalar.activation(hid[:], hid_ps[:], ACTF.Lrelu,
                                 bias=ec1_b1, alpha=0.01)
            m_ps = seq_ps(P, N)
            mmf(m_ps[:], ec1_w2, hid[:], start=True, stop=True)
            if l == 0:
                nc.vector.tensor_scalar(mx[:], m_ps[:], ec1_b2, None,
                                        op0=ALU.add)
            else:
                nc.vector.scalar_tensor_tensor(mx[:], m_ps[:], ec1_b2, mx[:],
                                               op0=ALU.add, op1=ALU.max)
        yT = pyT.tile([P, N], F32, tag="yT", name="yT")
        lrelu_into(yT[:], mx[:], accum=zpack[g][:, 6:7])
        if g == 0 and dbg is not None and 'dbgY' in dbg:
            dma(dbg['dbgY'][0], yT[:])
        nc.vector.tensor_reduce(zpack[g][:, 9:10], yT[:], axis=AXX, op=ALU.max)

        # ---- EC2 / EC3 ----
        for f in range(2):
            y2 = pwork.tile([P, N], F32, tag="y2", name="y2")
            nc.vector.tensor_scalar(y2[:], yT[:], 2.0, 0.0, op0=ALU.mult)
            nysq = pwork.tile([P, N], F32, tag="nysq", name="nysq")
            nc.vector.scalar_tensor_tensor(nysq[:], yT[:], -1.0, yT[:],
                                           op0=ALU.mult, op1=ALU.mult)
            gy_ps = [quad_ps() for _ in range(NT)]
            for t in range(NT):
                nc.tensor.matmul(gy_ps[t][:], y2[:, 128 * t:128 * (t + 1)],
                                 yT[:], start=True, stop=False)
                nc.tensor.matmul(gy_ps[t][:], allones[:], nysq[:],
                                 start=False, stop=True)
            Q2 = []
            for t in range(NT):
                q2 = pq2.tile([P, N], F32, tag=f"Q2{t}", name=f"Q2{t}")
                nc.scalar.copy(q2[:], gy_ps[t][:])
                nc.vector.scalar_tensor_tensor(
                    q2[:, 128 * t:128 * (t + 1)], eye, -1e30,
                    q2[:, 128 * t:128 * (t + 1)], op0=ALU.mult, op1=ALU.add)
                Q2.append(q2)
            gfT = []
            for t in range(NT):
                gps = quad_ps(P, P)
                nc.tensor.matmul(gps[0:P, 0:P], yT[:, 128 * t:128 * (t + 1)],
                                 ecf_g[f], start=True, stop=True)
                gt = phn.tile([P, P], F32, tag="gT", name="gT")
                nc.scalar.copy(gt[:], gps[0:P, 0:P])
                gfT.append(gt)
            af_ps = seq_ps(P, N)
            nc.tensor.matmul(af_ps[:], ecf_a[f], yT[:], start=True, stop=True)

            ts2 = topk3(Q2, lo=0)
            mx2 = pwork.tile([P, N], F32, tag="mx2", name="mx2")
            for l in range(3):
                gps = quad_ps()
                sel_gather(ts2, l, gfT, gps)
                if l == 0:
                    nc.vector.tensor_copy(mx2[:], gps[:])
                else:
                    nc.vector.tensor_tensor(mx2[:], mx2[:], gps[:], op=ALU.max)
            nc.vector.scalar_tensor_tensor(mx2[:], af_ps[:], ecf_b[f], mx2[:],
                                           op0=ALU.add, op1=ALU.add)
            yT_new = pyT.tile([P, N], F32, tag="yT", name="yT")
            lrelu_into(yT_new[:], mx2[:], accum=zpack[g][:, 7 + f:8 + f])
            if g == 0 and dbg is not None and 'dbgY' in dbg:
                dma(dbg['dbgY'][1 + f], yT_new[:])
            nc.vector.tensor_reduce(zpack[g][:, 10 + f:11 + f], yT_new[:],
                                    axis=AXX, op=ALU.max)
            yT = yT_new

        # ---- pools -> z0 row ----
        means_a = zpack[g][:].rearrange("p (a b) -> p a b", a=6, b=2)[:, 0:3, 0:1]
        nc.vector.tensor_scalar(means_a, means_a, 1.0 / N, 0.0, op0=ALU.mult)
        nc.vector.tensor_scalar(zpack[g][:, 6:9], zpack[g][:, 6:9], 1.0 / N, 0.0,
                                op0=ALU.mult)
        zr_ps = seq_ps(12, P)
        nc.tensor.transpose(zr_ps[0:12, 0:P], zpack[g][:], eye)
        zrow = pwork.tile([12, P], F32, tag="zrow", name="zrow")
        nc.scalar.copy(zrow[:], zr_ps[0:12, 0:P])
        dma(io['z0loc'][g].rearrange("(a b) -> a b", a=12), zrow[:])

    if skip_head:
        return

    # ---- head ----
    cores = list(range(NCORES))
    nc.gpsimd.collective_compute(
        "AllGather", ALU.bypass, replica_groups=[cores],
        ins=[io['z0loc'][:]], outs=[io['z0all'][:]])
    z0s = pst.tile([32, DIM2], F32, tag="z0s", name="z0s", padded_shape=[128, DIM2])
    dma(z0s[:], io['z0all'][:])
    tc.strict_bb_all_engine_barrier()
    zT = []
    spack = pst.tile([P, 12], F32, tag="spack", name="spack")
    qpack = pst.tile([P, 12], F32, tag="qpack", name="qpack")
    for t in range(12):
        zt_ps = seq_ps(P, 32)
        nc.tensor.transpose(zt_ps[0:P, 0:32], z0s[:, 128 * t:128 * (t + 1)],
                            eye[0:32, 0:32])
        zt = pst.tile([P, 32], F32, tag=f"zT{t}", name=f"zT{t}")
        nc.scalar.copy(zt[:], zt_ps[0:P, 0:32])
        zT.append(zt)
        nc.vector.tensor_reduce(spack[:, t:t + 1], zt[:], axis=AXX, op=ALU.add)
        ja = pjk.tile([P, N], F32, tag="junka", name="junka")
        nc.scalar.activation(ja[:, 0:32], zt[:], ACTF.Square,
                             accum_out=qpack[:, t:t + 1])
    mu = pst.tile([P, 12], F32, tag="mu", name="mu")
    nc.vector.tensor_scalar(mu[:], spack[:], 1.0 / 32, 0.0, op0=ALU.mult)
    m2 = pst.tile([P, 12], F32, tag="m2", name="m2")
    nc.vector.tensor_tensor(m2[:], mu[:], mu[:], op=ALU.mult)
    var = pst.tile([P, 12], F32, tag="var", name="var")
    nc.vector.scalar_tensor_tensor(var[:], qpack[:], 1.0 / 32, m2[:],
                                   op0=ALU.mult, op1=ALU.subtract)
    eps_col = pst.tile([P, 1], F32, tag="eps_col", name="eps_col")
    nc.any.memset(eps_col[:], 1e-5)
    sd = pst.tile([P, 12], F32, tag="sd", name="sd")
    nc.scalar.activation(sd[:], var[:], ACTF.Sqrt, bias=eps_col[:])
    inv = pst.tile([P, 12], F32, tag="inv", name="inv")
    nc.vector.reciprocal(inv[:], sd[:])
    gam = wslice('bn_scale')
    bet = wslice('bn_shift')
    sc = pst.tile([P, 12], F32, tag="sc", name="sc")
    nc.vector.tensor_tensor(sc[:], inv[:], gam, op=ALU.mult)
    bi = pst.tile([P, 12], F32, tag="bi", name="bi")
    nc.vector.tensor_tensor(bi[:], mu[:], sc[:], op=ALU.mult)
    nc.vector.tensor_tensor(bi[:], bet, bi[:], op=ALU.subtract)
    zcur = []
    for t in range(12):
        zc = pst.tile([P, 32], F16, tag=f"zc{t}", name=f"zc{t}")
        nc.scalar.activation(zc[:], zT[t][:], ACTF.Identity,
                             bias=bi[:, t:t + 1], scale=sc[:, t:t + 1])
        zcur.append(zc[:])

    linBc = wslice('linBcol')
    for li in range(LIN_D):
        wt = wts[li]
        psA = seq_ps(P, 32)
        psB = seq_ps(64, 32)
        for k in range(12):
            wa = wt[:, HSH * k:HSH * k + 128]
            wb = wt[:, HSH * k + 128:HSH * (k + 1)]
            nc.tensor.matmul(psA[:], wa, zcur[k], start=(k == 0),
                             stop=(k == 11))
            nc.tensor.matmul(psB[:], wb, zcur[k], start=(k == 0),
                             stop=(k == 11))
        zslA = pwork.tile([P, 32], F16, tag="zslA", name="zslA")
        zslB = pwork.tile([64, 32], F16, tag="zslB", name="zslB")
        nc.scalar.activation(zslA[:], psA[:], ACTF.Lrelu,
                             bias=linBc[:, 2 * li:2 * li + 1], alpha=0.01)
        nc.scalar.activation(zslB[:], psB[0:64, :], ACTF.Lrelu,
                             bias=linBc[0:64, 2 * li + 1:2 * li + 2],
                             alpha=0.01)
        dma(io['zsl'][li][0:128, :], zslA[:])
        dma(io['zsl'][li][128:192, :], zslB[:])
        nc.gpsimd.collective_compute(
            "AllGather", ALU.bypass, replica_groups=[cores],
            ins=[io['zsl'][li][:]], outs=[io['zfull'][li][:]])
        zcat = pst.tile([P, 12 * 32], F16, tag=f"zcat{li}", name=f"zcat{li}")
        dma(zcat[:].rearrange("p (k b) -> p k b", k=12),
            io['zfull'][li].rearrange("(k p) b -> p k b", k=12))
        zcur = [zcat[:, 32 * t:32 * (t + 1)] for t in range(12)]
    tc.strict_bb_all_engine_barrier()
    outW_sb = wslice('outW')
    outWh = pwork.tile([P, 12], F16, tag="outWh", name="outWh")
    nc.vector.tensor_copy(outWh[:], outW_sb)
    outb_sb = wslice('outb', 1)
    ps_out = seq_ps(1, 32)
    for k in range(12):
        nc.tensor.matmul(ps_out[:], outWh[:, k:k + 1], zcur[k],
                         start=(k == 0), stop=(k == 11))
    o32 = pwork.tile([1, 32], F32, tag="o32", name="o32")
    nc.scalar.activation(o32[:], ps_out[:], ACTF.Identity, bias=outb_sb)
    dma(io['out32'][:], o32[:])


def build_nc(skip_head=False):
    nc = bacc.Bacc("TRN2", target_bir_lowering=False, debug=False,
                   num_devices=NCORES)
    io = {}

    def inp(name, shape):
        io[name] = nc.dram_tensor(name, list(shape), F32, kind="ExternalInput").ap()

    inp('xpack', (GPC, 128, 2 * N + 24))
    inp('wpack', (128, WPACK_COLS))
    io['linW'] = nc.dram_tensor("linW", [LIN_D, 12, 128, HSH], F16,
                                kind="ExternalInput").ap()

    io['z0loc'] = nc.dram_tensor(
        "z0loc", [GPC, DIM2], F32,
        kind="ExternalOutput" if skip_head else "Internal").ap()
    io['out32'] = nc.dram_tensor("out32", [1, 32], F32,
                                 kind="ExternalOutput").ap()
    if not skip_head:
        io['z0all'] = nc.dram_tensor("z0all", [B, DIM2], F32,
                                     addr_space="Shared").ap()
        io['zsl'] = [nc.dram_tensor(f"zsl{li}", [HSH, 32], F16).ap()
                     for li in range(LIN_D)]
        io['zfull'] = [nc.dram_tensor(f"zfull{li}", [DIM2, 32], F16,
                                      addr_space="Shared").ap()
                       for li in range(LIN_D)]

    with tile.TileContext(nc) as tc:
        core_program(tc, io, skip_head=skip_head)
    nc.compile()
    return nc


_CACHED = {}
_DEBUG_IO = None


def kernel(**inputs) -> np.ndarray:
    from concourse.bass_utils import run_bass_kernel_spmd
    if 'nc' not in _CACHED:
        _CACHED['nc'] = build_nc()
    nc = _CACHED['nc']
    in_maps = []
    for c in range(NCORES):
        d = prep_host(inputs, c)
        in_maps.append({k: np.ascontiguousarray(v) for k, v in d.items()})
    res = run_bass_kernel_spmd(nc, in_maps, core_ids=list(range(NCORES)),
                               trace=bool(os.environ.get("KBENCH_TRACE")))
    _CACHED['last'] = res
    return res.results[0]['out32'].reshape(-1).astype(np.float32)


if __name__ == "__main__":
    data = dict(np.load('/root/problem/inputs.npz'))
    out = kernel(**data)
    print("kernel out:", out[:5])



# revision 63
# speedup vs baseline: 1.0286x; 1.0286x over previous
"""Trainium2 Bass kernel for nn_EnsembleNet3 (gnn_message_passing).

Self-contained: takes full inputs (as produced by setup_inputs()), shards the
B=32 graph dim over 8 NeuronCores (4 graphs/core), runs the graph stack fully
on-device, and computes the [B,1536] head sharded over output columns with
AllGathers between layers.

Per graph (N=512 nodes):
- kNN-100 for TAGConv: per-row threshold via count-secant iteration on
  Q[i,j] = 2*G[i,j] - n_j (same per-row order as -dist; Q row max is self),
  finished by an exact masked max-8 endgame; adjacency applied as a dense 0/1
  mask matmul on PE with host-folded hop weights:
  out = h@W~0 + (Mh)@W~1 + (M^2 h)@W~2,  M = mask incl self.
- EdgeConv k=3: top-3 indices via max8+max_index on Q; gather done as an
  exact one-hot selection matmul on PE: Sel_t[j,i] = (idx_l(i) == 128t+j)
  built by DVE compares from a PE-replicated index row, then
  gathered = g^T~t . Sel_t accumulated over tiles in PSUM. MLPs decomposed
  as a_i + g_j so only g is gathered; max-aggregation commutes with
  leaky_relu. leaky_relu computed as max(x, 0.01x) in one fused DVE op
  (with free mean-pool accumulation) or via ACT Lrelu with bias folding.
"""
import os
from contextlib import ExitStack

import numpy as np

import concourse.bass as bass
import concourse.bacc as bacc
import concourse.tile as tile
from concourse import mybir
from concourse._compat import with_exitstack

F32 = mybir.dt.float32
F32R = mybir.dt.float32r
F16 = mybir.dt.float16
U16 = mybir.dt.uint16
U32 = mybir.dt.uint32
U8 = mybir.dt.uint8
ALU = mybir.AluOpType
ACTF = mybir.ActivationFunctionType
AXX = mybir.AxisListType.X

B, N, F, W = 32, 512, 6, 128
NT = N // 128
GPC = 4
NCORES = 8
K101 = 101
SEL_ITERS = 8
SEL_TARGET = float(K101 + 4)
U_LO, U_HI = -16.0, 16.0
DIM2 = 1536
HSH = DIM2 // NCORES
LIN_D = 5
DVE_COLS = 7    # selection count passes: cols < DVE_COLS on DVE, rest on ACT


def _fold_tag(Wk, b):
    W0, W1, W2 = Wk[0], Wk[1], Wk[2]
    c1, c2 = 1.0 / 100.0, 1.0 / 10000.0
    return (
        (W0 - W1 * c1 + W2 * c2).astype(np.float32),
        (W1 * c1 - 2.0 * W2 * c2).astype(np.float32),
        (W2 * c2).astype(np.float32),
        b.astype(np.float32),
    )


def prep_host(inputs, core):
    inp = {k: np.asarray(v) for k, v in inputs.items()}
    x = inp['x'].astype(np.float32).reshape(B, N, F)
    xs = x[GPC * core:GPC * (core + 1)]
    f32 = np.float32

    # --- per-graph input pack [128, 1048]: xgT | xgR | xnm ---
    xt = xs.transpose(0, 2, 1)
    xpack = np.zeros((GPC, 128, 2 * N + 24), f32)
    xpack[:, 0:F, 0:N] = xt
    xpack[:, F:2 * F, 0:N] = 1.0
    xpack[:, 0:F, N:2 * N] = 2.0 * xt
    xpack[:, F:2 * F, N:2 * N] = -(xt * xt)
    xpack[:, 12, N:2 * N] = 1.0
    for t in range(NT):
        xpack[:, :, 2 * N + F * t:2 * N + F * (t + 1)] = xs[:, 128 * t:128 * (t + 1), :]

    # --- const pack [128, cols] ---
    cols = {}
    blobs = []
    off = 0

    def put(name, arr2d):
        nonlocal off
        a = np.asarray(arr2d, f32)
        pad = np.zeros((128, a.shape[1]), f32)
        pad[:a.shape[0]] = a
        cols[name] = (off, a.shape[1])
        blobs.append(pad)
        off += a.shape[1]

    put('eye', np.eye(128, dtype=f32))
    put('iota8', np.broadcast_to(np.arange(8, dtype=f32), (128, 8)))
    put('iotaP', np.arange(128, dtype=f32).reshape(128, 1))
    rsel = np.zeros((3, 384), f32)
    for l in range(3):
        rsel[l, 128 * l:128 * (l + 1)] = 1.0
    put('rsel3', rsel)
    for li, (Wk, bk) in enumerate(
            [(inp['tag1_W'], inp['tag1_b']),
             (inp['tag_W'][0], inp['tag_b'][0]),
             (inp['tag_W'][1], inp['tag_b'][1])]):
        w0, w1, w2, bb = _fold_tag(Wk, bk)
        put(f'tagw{li}', np.concatenate([w0, w1, w2], axis=1))
        put(f'tagb{li}', bb.reshape(128, 1))
    W1 = inp['p1_W1'].astype(f32)
    put('ec1_a', W1[:F] - W1[F:])
    put('ec1_g', W1[F:])
    put('ec1_b1', inp['p1_b1'].astype(f32).reshape(128, 1))
    put('ec1_w2', inp['p1_W2'].astype(f32))
    put('ec1_b2', inp['p1_b2'].astype(f32).reshape(128, 1))
    for f in range(2):
        Wf = inp['pf_W'][f].astype(f32)
        put(f'ec{f+2}_a', Wf[:W] - Wf[W:])
        put(f'ec{f+2}_g', Wf[W:])
        put(f'ec{f+2}_b', inp['pf_b'][f].astype(f32).reshape(128, 1))
    put('bn_scale', inp['bn_gamma'].astype(f32).reshape(12, 128).T)
    put('bn_shift', inp['bn_beta'].astype(f32).reshape(12, 128).T)
    put('outW', inp['out_W'].astype(f32).reshape(12, 128).T)
    put('outb', np.full((1, 1), float(inp['out_b'][0]), f32))
    sl = slice(HSH * core, HSH * (core + 1))
    put('linB', inp['lin_b'].astype(f32)[:, sl].reshape(1, LIN_D * HSH))
    lbc = np.zeros((128, 2 * LIN_D), f32)
    for li in range(LIN_D):
        lb = inp['lin_b'].astype(f32)[li, sl]
        lbc[0:128, 2 * li] = lb[0:128]
        lbc[0:64, 2 * li + 1] = lb[128:192]
    put('linBcol', lbc)
    owc = np.zeros((128, 2), f32)
    ow = inp['out_W'].astype(f32).reshape(-1)[sl]
    owc[0:128, 0] = ow[0:128]
    owc[0:64, 1] = ow[128:192]
    put('outWc', owc)
    wpack = np.concatenate(blobs, axis=1)
    assert wpack.shape[1] == WPACK_COLS, (wpack.shape, off)
    assert all(cols[k] == WOFF[k] for k in cols), "WOFF mismatch"

    linW = inp['lin_W'].astype(np.float16)
    d = {
        'xpack': np.ascontiguousarray(xpack),
        'wpack': np.ascontiguousarray(wpack),
        'linW': np.ascontiguousarray(linW[:, :, sl].reshape(LIN_D, 12, 128, HSH)),
    }
    return d


def _woff_table():
    off = 0
    tab = {}
    def put(name, w):
        nonlocal off
        tab[name] = (off, w)
        off += w
    put('eye', 128); put('iota8', 8); put('iotaP', 1); put('rsel3', 384)
    for li in range(3):
        put(f'tagw{li}', 384); put(f'tagb{li}', 1)
    put('ec1_a', 128); put('ec1_g', 128); put('ec1_b1', 1)
    put('ec1_w2', 128); put('ec1_b2', 1)
    for f in range(2):
        put(f'ec{f+2}_a', 128); put(f'ec{f+2}_g', 128); put(f'ec{f+2}_b', 1)
    put('bn_scale', 12); put('bn_shift', 12)
    put('outW', 12); put('outb', 1)
    put('linB', LIN_D * HSH); put('linBcol', 2 * LIN_D); put('outWc', 2)
    return tab, off


WOFF, WPACK_COLS = _woff_table()


@with_exitstack
def core_program(ctx: ExitStack, tc: tile.TileContext, io: dict, skip_head=False):
    nc = tc.nc
    P = 128

    const = ctx.enter_context(tc.tile_pool(name="const", bufs=1))
    pq = ctx.enter_context(tc.tile_pool(name="pq", bufs=16))
    pq2 = ctx.enter_context(tc.tile_pool(name="pq2", bufs=1))
    pmask = ctx.enter_context(tc.tile_pool(name="pmask", bufs=8))
    pwork = ctx.enter_context(tc.tile_pool(name="pwork", bufs=1))
    psel = ctx.enter_context(tc.tile_pool(name="psel", bufs=8))
    phid = ctx.enter_context(tc.tile_pool(name="phid", bufs=2))
    phT = ctx.enter_context(tc.tile_pool(name="phT", bufs=4))
    pyT = ctx.enter_context(tc.tile_pool(name="pyT", bufs=3))
    phn = ctx.enter_context(tc.tile_pool(name="phn", bufs=8))
    pst = ctx.enter_context(tc.tile_pool(name="pst", bufs=1))
    phw = ctx.enter_context(tc.tile_pool(name="phw", bufs=1))
    psq = ctx.enter_context(tc.tile_pool(name="psq", bufs=5, space="PSUM"))
    pss = ctx.enter_context(tc.tile_pool(name="pss", bufs=3, space="PSUM"))

    def quad_ps(pp=P, nn=N):
        return psq.tile([pp, nn], F32, tag="quad", name="quad")

    def seq_ps(pp, nn):
        return pss.tile([pp, nn], F32, tag="seq", name="seq")

    def dma(dst, src):
        nc.sync.dma_start(dst, src)

    # All value matmuls are exact fp32: the head BatchNorm divides pooled
    # features by ~3e-3 (tiny batch variance), amplifying any value-path
    # rounding ~300x; f32r's 2.4e-4 rounding would blow the 2e-2 gate.
    # f32r is used ONLY where operands are exact small ints / 0-1 (rep).
    def mmf(out, lhsT, rhs, start, stop):
        nc.tensor.matmul(out, lhsT, rhs, start=start, stop=stop)

    def mmr(out, lhsT, rhs, start, stop):
        # f32r: 1 cyc/row vs fp32's 4. Only on paths that tolerate the
        # ~2.4e-4 operand rounding (gather payloads, 2nd-hop aggregates);
        # never on x/y/weights that feed rankings or the pools directly.
        nc.tensor.matmul(out, lhsT.bitcast(F32R), rhs.bitcast(F32R),
                         start=start, stop=stop)

    # ---- constants: one packed DMA ----
    # DMA'd with f32r dtype so the weights count as f32r-rounded producers
    # for the walrus verifier (bytes unchanged; PE truncates on read).
    wp = const.tile([P, WPACK_COLS], F32, tag="wpack", name="wpack")
    dma(wp[:], io['wpack'][:])

    def wslice(name, rows=128):
        o, w = WOFF[name]
        return wp[0:rows, o:o + w]

    eye = wslice('eye')
    iota8 = wslice('iota8')
    iotaP = wslice('iotaP')
    rsel3 = wslice('rsel3', 3)
    ones_col = const.tile([P, 1], F32)
    nc.any.memset(ones_col[:], 1.0)
    ones32 = const.tile([1, 32], F32, padded_shape=[128, 32])
    nc.any.memset(ones32[:], 1.0)
    allones = const.tile([P, P], F32)
    nc.any.memset(allones[:], 1.0)

    tagw, tagb = [], []
    for li in range(3):
        fin = F if li == 0 else W
        wt = wslice(f'tagw{li}', fin)
        tagw.append([wt[:, 128 * k:128 * (k + 1)] for k in range(3)])
        tagb.append(wslice(f'tagb{li}'))

    rs_o, rs_w = WOFF['rsel3']
    rsel3r_t = const.tile([P, rs_w], F32, tag="r_rsel3", name="r_rsel3")
    nc.vector.tensor_copy(rsel3r_t[:].bitcast(F32R), wp[0:P, rs_o:rs_o + rs_w])
    rsel3r = rsel3r_t[0:3, :]

    ec1_a = wslice('ec1_a', F)
    ec1_g = wslice('ec1_g', F)
    ec1_w2 = wslice('ec1_w2')
    ec1_b1 = wslice('ec1_b1')
    ec1_b2 = wslice('ec1_b2')
    ecf_a = [wslice('ec2_a'), wslice('ec3_a')]
    ecf_g = [wslice('ec2_g'), wslice('ec3_g')]
    ecf_b = [wslice('ec2_b'), wslice('ec3_b')]

    # ---- inputs per graph: one packed DMA each ----
    xgT, xgR, xnm = [], [], []
    for g in range(GPC):
        xp = pst.tile([P, 2 * N + 24], F32, tag=f"xpack{g}", name=f"xpack{g}")
        dma(xp[:], io['xpack'][g])
        xgT.append(xp[:, 0:N])
        xgR.append(xp[:, N:2 * N])
        xnm.append([xp[:, 2 * N + F * t:2 * N + F * (t + 1)] for t in range(NT)])

    # ---- prefetch head weights early (fp16): streams during graph phase ----
    wts = []
    for li in range(LIN_D):
        wt = phw.tile([P, 12 * HSH], F16, tag=f"linW{li}", name=f"linW{li}")
        dma(wt[:].rearrange("a (k b) -> a k b", k=12),
            io['linW'][li].rearrange("k a b -> a k b"))
        wts.append(wt)

    # ---- Q = 2G - n_row via augmented matmul (K=12), n_col from x_nm ----
    Q = [[None] * NT for _ in range(GPC)]
    for g in range(GPC):
        gps = [quad_ps() for _ in range(NT)]
        for t in range(NT):
            nc.tensor.matmul(gps[t][:], xgT[g][0:12, 128 * t:128 * (t + 1)],
                             xgR[g][0:12, 0:N], start=True, stop=True)
        for t in range(NT):
            qt = pq.tile([P, N], F32, tag="Q", name="Q")
            nc.scalar.copy(qt[:], gps[t][:])
            Q[g][t] = qt

    # ---- per-graph count-secant selection (pipelines across graphs) ----
    # DVE cols [0, SEL_DVE), ACT cols [SEL_DVE, NT) per graph per iteration.
    SEL_DVE = 2
    pjk = ctx.enter_context(tc.tile_pool(name="pjk", bufs=2))

    def sel_graph(g):
        """Exact 101st-largest threshold per Q row -> ustar_g [P, NT]."""
        def t4(nm, init=None):
            tt = pst.tile([P, NT], F32, tag=f"{nm}{g}", name=f"{nm}{g}")
            if init is not None:
                nc.any.memset(tt[:], init)
            return tt
        st_u = t4('su', U_HI + (U_LO - U_HI) * (SEL_TARGET / N))
        st_ul = t4('sul', U_HI)
        st_uh = t4('suh', U_LO)
        st_cl = t4('scl', 0.0)
        st_ch = t4('sch', float(N))
        cnt = t4('scn')
        tmp_a = t4('sta')
        tmp_b = t4('stb')
        tmp_m = pst.tile([P, NT], U8, tag=f"stm{g}", name=f"stm{g}")
        for it in range(SEL_ITERS):
            for t in range(NT):
                ucol = st_u[:, t:t + 1]
                ccol = cnt[:, t:t + 1]
                if t < SEL_DVE:
                    jd = pjk.tile([P, N], F32, tag="junkd", name="junkd")
                    nc.vector.tensor_scalar(
                        jd[:], Q[g][t][:], ucol, 0.0,
                        op0=ALU.is_ge, op1=ALU.add, accum_out=ccol)
                else:
                    ja = pjk.tile([P, N], F32, tag="junka", name="junka")
                    nc.scalar.activation(
                        ja[:], Q[g][t][:], ACTF.Sign,
                        bias=ucol, scale=-1.0, accum_out=ccol)
            # ACT cols report sum(sign): c = 256 - s/2
            nc.vector.tensor_scalar(
                cnt[:, SEL_DVE:NT], cnt[:, SEL_DVE:NT], -0.5, 256.0,
                op0=ALU.mult, op1=ALU.add)
            nc.vector.tensor_scalar(
                tmp_m[:], cnt[:], float(K101) - 0.5, 0.0, op0=ALU.is_ge)
            nc.vector.copy_predicated(st_uh[:], tmp_m[:], st_u[:])
            nc.vector.copy_predicated(st_ch[:], tmp_m[:], cnt[:])
            nc.vector.tensor_scalar(
                tmp_m[:], cnt[:], float(K101) - 0.5, 0.0, op0=ALU.is_lt)
            nc.vector.copy_predicated(st_ul[:], tmp_m[:], st_u[:])
            nc.vector.copy_predicated(st_cl[:], tmp_m[:], cnt[:])
            if it == SEL_ITERS - 1:
                break
            nc.vector.tensor_tensor(tmp_a[:], st_ch[:], st_cl[:],
                                    op=ALU.subtract)
            nc.vector.reciprocal(tmp_a[:], tmp_a[:])
            nc.vector.scalar_tensor_tensor(
                tmp_b[:], st_ch[:], -SEL_TARGET, tmp_a[:],
                op0=ALU.add, op1=ALU.mult)
            nc.vector.tensor_scalar(
                tmp_b[:], tmp_b[:], 0.05, 0.95, op0=ALU.max, op1=ALU.min)
            nc.vector.tensor_tensor(tmp_a[:], st_ul[:], st_uh[:],
                                    op=ALU.subtract)
            nc.vector.tensor_tensor(tmp_a[:], tmp_a[:], tmp_b[:], op=ALU.mult)
            nc.vector.tensor_tensor(st_u[:], st_uh[:], tmp_a[:], op=ALU.add)
        # endgame: exact 101st-largest via masked top-8
        ustar_g = t4('sus')
        pos_g = t4('sps')
        nc.vector.tensor_scalar(pos_g[:], st_ch[:], -float(K101), 0.0,
                                op0=ALU.add)
        for t in range(NT):
            zm = pwork.tile([P, N], F32, tag="zm", name="zm")
            nc.vector.tensor_scalar(
                zm[:], Q[g][t][:], st_uh[:, t:t + 1], -1e30,
                op0=ALU.is_lt, op1=ALU.mult)
            nc.vector.tensor_tensor(zm[:], zm[:], Q[g][t][:], op=ALU.subtract)
            m8 = pwork.tile([P, 8], F32, tag="m8e", name="m8e")
            nc.vector.max(m8[:], zm[:])
            msk8 = pwork.tile([P, 8], F32, tag="msk8", name="msk8")
            nc.vector.tensor_tensor(
                msk8[:], iota8,
                pos_g[:, t:t + 1].broadcast_to([P, 8]), op=ALU.is_equal)
            j8 = pwork.tile([P, 8], F32, tag="j8", name="j8")
            nc.vector.scalar_tensor_tensor(
                j8[:], m8[:], -1.0, msk8[:], op0=ALU.mult, op1=ALU.mult,
                accum_out=ustar_g[:, t:t + 1])
        return ustar_g

    lrelu_op = dict(op0=ALU.mult, op1=ALU.max)

    def lrelu_into(dst, src, accum=None):
        nc.vector.scalar_tensor_tensor(dst, src, 0.01, src, accum_out=accum,
                                       **lrelu_op)

    def topk3(Qt, lo=1):
        """top-3 neighbor indices per node -> ts3s [3, N] fp32.

        Takes ranks lo..lo+2. lo=1 skips rank 0 (self, when Q row max is
        self); lo=0 is used when the diagonal is already masked out.
        """
        ts3 = seq_ps(3, N)
        for t in range(NT):
            m8 = pwork.tile([P, 8], F32, tag="m8g", name="m8g")
            nc.vector.max(m8[:], Qt[t][:])
            i8 = pwork.tile([P, 8], U32, tag="i8g", name="i8g")
            nc.vector.max_index(i8[:], m8[:], Qt[t][:])
            i8f = pwork.tile([P, 8], F32, tag="i8f", name="i8f")
            nc.vector.tensor_copy(i8f[:], i8[:])
            nc.tensor.transpose(ts3[0:3, 128 * t:128 * (t + 1)],
                                i8f[:, lo:lo + 3], eye)
        ts3s = pwork.tile([3, N], F32, tag="ts3s", name="ts3s", padded_shape=[128, N])
        nc.scalar.copy(ts3s[:].bitcast(F32R), ts3[0:3, :])
        return ts3s

    def sel_gather(ts3s, l, lhsTs, ps, close=True):
        """ps[f, i] (+)= payload[f, idx_l(i)] via one-hot Sel matmuls.

        Sel_t[j, i] = (idx_l(i) == 128t + j), built by DVE compare from a
        PE-replicated index row; lhsTs[t] = node-major payload [j, f].
        """
        rep_ps = seq_ps(P, N)
        nc.tensor.matmul(rep_ps[:],
                         rsel3r[0:3, 128 * l:128 * (l + 1)].bitcast(F32R),
                         ts3s[0:3, 0:N].bitcast(F32R), start=True, stop=True)
        for t in range(NT):
            sel = psel.tile([P, N], F32, tag="sel", name="sel")
            nc.vector.tensor_scalar(
                sel[:], rep_ps[:], iotaP, float(128 * t),
                op0=ALU.subtract, op1=ALU.is_equal)
            mmf(ps[:], lhsTs[t][:], sel[:], start=(t == 0),
                stop=(close and t == NT - 1))

    zpack = [pst.tile([P, 12], F32, tag=f"zpack{g}", name=f"zpack{g}") for g in range(GPC)]
    dbg = globals().get('_DEBUG_IO')

    for g in range(GPC):
        ustar_g = sel_graph(g)
        # ---- maskT: M[i,j] = (Q[i,j] >= u*_i) on DVE, then PE transposes ----
        Mrows = []
        for t in range(NT):
            mrow = pwork.tile([P, N], F32, tag="Mrow", name="Mrow")
            nc.vector.tensor_scalar(
                mrow[:], Q[g][t][:], ustar_g[:, t:t + 1], 0.0,
                op0=ALU.is_ge)
            Mrows.append(mrow)
        maskT = []
        for jc in range(NT):
            mps = quad_ps()
            for it in range(NT):
                nc.tensor.transpose(mps[0:P, 128 * it:128 * (it + 1)],
                                    Mrows[it][:, 128 * jc:128 * (jc + 1)], eye)
            mt = pmask.tile([P, N], F32, tag="maskT", name="maskT")
            nc.scalar.copy(mt[:], mps[:])
            maskT.append(mt)

        # ---- TAG ----
        hT = xgT[g][0:F, 0:N]
        hnm = xnm[g]
        for li in range(3):
            fin = F if li == 0 else W
            u1n_ps = [quad_ps(P, fin) for _ in range(NT)]
            for ic in range(NT):
                for jc in range(NT):
                    mmf(u1n_ps[ic][0:P, 0:fin],
                        maskT[jc][:, 128 * ic:128 * (ic + 1)],
                        hnm[jc][:], start=(jc == 0), stop=(jc == NT - 1))
            u1n = []
            for ic in range(NT):
                uu = phn.tile([P, fin], F32, tag="u1n", name="u1n")
                nc.scalar.copy(uu[:], u1n_ps[ic][0:P, 0:fin])
                u1n.append(uu)
            u1T_ps = seq_ps(fin, N)
            for ic in range(NT):
                nc.tensor.transpose(u1T_ps[0:fin, 128 * ic:128 * (ic + 1)],
                                    u1n[ic][:], eye)
            u1T = pwork.tile([fin, N], F32, tag="u1T", name="u1T", padded_shape=[128, N])
            nc.scalar.copy(u1T[:], u1T_ps[0:fin, :])
            u2T_ps = seq_ps(fin, N)
            for jc in range(NT):
                mmf(u2T_ps[0:fin, :], u1n[jc][:], maskT[jc][:],
                    start=(jc == 0), stop=(jc == NT - 1))
            u2T = pwork.tile([fin, N], F32, tag="u2T", name="u2T", padded_shape=[128, N])
            nc.scalar.copy(u2T[:], u2T_ps[0:fin, :])
            oT_ps = seq_ps(P, N)
            nc.tensor.matmul(oT_ps[:], tagw[li][0], hT[:], start=True, stop=False)
            nc.tensor.matmul(oT_ps[:], tagw[li][1], u1T[:], start=False, stop=False)
            nc.tensor.matmul(oT_ps[:], tagw[li][2], u2T[:], start=False, stop=True)
            sT = pwork.tile([P, N], F32, tag="sT", name="sT")
            nc.scalar.activation(sT[:], oT_ps[:], ACTF.Identity, bias=tagb[li])
            hT_new = phT.tile([P, N], F32, tag="hT", name="hT")
            lrelu_into(hT_new[:], sT[:],
                       accum=zpack[g][:, 2 * li:2 * li + 1])
            nc.vector.tensor_reduce(zpack[g][:, 2 * li + 1:2 * li + 2], hT_new[:],
                                    axis=AXX, op=ALU.max)
            if g == 0 and dbg is not None and 'dbgH' in dbg:
                dma(dbg['dbgH'][li], hT_new[:])
            hT = hT_new
            if li < 2:
                hnm = []
                for t in range(NT):
                    hps = quad_ps(P, P)
                    nc.tensor.transpose(hps[0:P, 0:P], hT[:, 128 * t:128 * (t + 1)],
                                        eye)
                    hh = phn.tile([P, P], F32, tag="hnm", name="hnm")
                    nc.scalar.copy(hh[:], hps[0:P, 0:P])
                    hnm.append(hh)

        # ---- EC1 ----
        g1T = []
        for t in range(NT):
            gps = quad_ps(P, P)
            nc.tensor.matmul(gps[0:P, 0:P], xgT[g][0:F, 128 * t:128 * (t + 1)],
                             ec1_g, start=True, stop=True)
            gt = phn.tile([P, P], F32, tag="gT", name="gT")
            nc.scalar.copy(gt[:], gps[0:P, 0:P])
            g1T.append(gt)
        ts1 = topk3(Q[g])
        mx = pwork.tile([P, N], F32, tag="mx", name="mx")
        for l in range(3):
            hid_ps = quad_ps()
            sel_gather(ts1, l, g1T, hid_ps, close=False)
            mmf(hid_ps[:], ec1_a, xgT[g][0:F, 0:N], start=False, stop=True)
            hid = phid.tile([P, N], F32, tag="hid", name="hid")
            nc.scalar.activation(hid[:], hid_ps[:], ACTF.Lrelu,
                                 bias=ec1_b1, alpha=0.01)
            m_ps = seq_ps(P, N)
            mmf(m_ps[:], ec1_w2, hid[:], start=True, stop=True)
            if l == 0:
                nc.vector.tensor_scalar(mx[:], m_ps[:], ec1_b2, None,
                                        op0=ALU.add)
            else:
                nc.vector.scalar_tensor_tensor(mx[:], m_ps[:], ec1_b2, mx[:],
                                               op0=ALU.add, op1=ALU.max)
        yT = pyT.tile([P, N], F32, tag="yT", name="yT")
        lrelu_into(yT[:], mx[:], accum=zpack[g][:, 6:7])
        if g == 0 and dbg is not None and 'dbgY' in dbg:
            dma(dbg['dbgY'][0], yT[:])
        nc.vector.tensor_reduce(zpack[g][:, 9:10], yT[:], axis=AXX, op=ALU.max)

        # ---- EC2 / EC3 ----
        for f in range(2):
            y2 = pwork.tile([P, N], F32, tag="y2", name="y2")
            nc.vector.tensor_scalar(y2[:], yT[:], 2.0, 0.0, op0=ALU.mult)
            nysq = pwork.tile([P, N], F32, tag="nysq", name="nysq")
            nc.vector.scalar_tensor_tensor(nysq[:], yT[:], -1.0, yT[:],
                                           op0=ALU.mult, op1=ALU.mult)
            gy_ps = [quad_ps() for _ in range(NT)]
            for t in range(NT):
                nc.tensor.matmul(gy_ps[t][:], y2[:, 128 * t:128 * (t + 1)],
                                 yT[:], start=True, stop=False)
                nc.tensor.matmul(gy_ps[t][:], allones[:], nysq[:],
                                 start=False, stop=True)
            Q2 = []
            for t in range(NT):
                q2 = pq2.tile([P, N], F32, tag=f"Q2{t}", name=f"Q2{t}")
                nc.scalar.copy(q2[:], gy_ps[t][:])
                nc.vector.scalar_tensor_tensor(
                    q2[:, 128 * t:128 * (t + 1)], eye, -1e30,
                    q2[:, 128 * t:128 * (t + 1)], op0=ALU.mult, op1=ALU.add)
                Q2.append(q2)
            gfT = []
            for t in range(NT):
                gps = quad_ps(P, P)
                nc.tensor.matmul(gps[0:P, 0:P], yT[:, 128 * t:128 * (t + 1)],
                                 ecf_g[f], start=True, stop=True)
                gt = phn.tile([P, P], F32, tag="gT", name="gT")
                nc.scalar.copy(gt[:], gps[0:P, 0:P])
                gfT.append(gt)
            af_ps = seq_ps(P, N)
            nc.tensor.matmul(af_ps[:], ecf_a[f], yT[:], start=True, stop=True)

            ts2 = topk3(Q2, lo=0)
            mx2 = pwork.tile([P, N], F32, tag="mx2", name="mx2")
            for l in range(3):
                gps = quad_ps()
                sel_gather(ts2, l, gfT, gps)
                if l == 0:
                    nc.vector.tensor_copy(mx2[:], gps[:])
                else:
                    nc.vector.tensor_tensor(mx2[:], mx2[:], gps[:], op=ALU.max)
            nc.vector.scalar_tensor_tensor(mx2[:], af_ps[:], ecf_b[f], mx2[:],
                                           op0=ALU.add, op1=ALU.add)
            yT_new = pyT.tile([P, N], F32, tag="yT", name="yT")
            lrelu_into(yT_new[:], mx2[:], accum=zpack[g][:, 7 + f:8 + f])
            if g == 0 and dbg is not None and 'dbgY' in dbg:
                dma(dbg['dbgY'][1 + f], yT_new[:])
            nc.vector.tensor_reduce(zpack[g][:, 10 + f:11 + f], yT_new[:],
                                    axis=AXX, op=ALU.max)
            yT = yT_new

        # ---- pools -> z0 row ----
        means_a = zpack[g][:].rearrange("p (a b) -> p a b", a=6, b=2)[:, 0:3, 0:1]
        nc.vector.tensor_scalar(means_a, means_a, 1.0 / N, 0.0, op0=ALU.mult)
        nc.vector.tensor_scalar(zpack[g][:, 6:9], zpack[g][:, 6:9], 1.0 / N, 0.0,
                                op0=ALU.mult)
        zr_ps = seq_ps(12, P)
        nc.tensor.transpose(zr_ps[0:12, 0:P], zpack[g][:], eye)
        zrow = pwork.tile([12, P], F32, tag="zrow", name="zrow")
        nc.scalar.copy(zrow[:], zr_ps[0:12, 0:P])
        dma(io['z0loc'][g].rearrange("(a b) -> a b", a=12), zrow[:])

    if skip_head:
        return

    # ---- head ----
    cores = list(range(NCORES))
    nc.gpsimd.collective_compute(
        "AllGather", ALU.bypass, replica_groups=[cores],
        ins=[io['z0loc'][:]], outs=[io['z0all'][:]])
    z0s = pst.tile([32, DIM2], F32, tag="z0s", name="z0s", padded_shape=[128, DIM2])
    dma(z0s[:], io['z0all'][:])
    tc.strict_bb_all_engine_barrier()
    zT = []
    spack = pst.tile([P, 12], F32, tag="spack", name="spack")
    qpack = pst.tile([P, 12], F32, tag="qpack", name="qpack")
    for t in range(12):
        zt_ps = seq_ps(P, 32)
        nc.tensor.transpose(zt_ps[0:P, 0:32], z0s[:, 128 * t:128 * (t + 1)],
                            eye[0:32, 0:32])
        zt = pst.tile([P, 32], F32, tag=f"zT{t}", name=f"zT{t}")
        nc.scalar.copy(zt[:], zt_ps[0:P, 0:32])
        zT.append(zt)
        nc.vector.tensor_reduce(spack[:, t:t + 1], zt[:], axis=AXX, op=ALU.add)
        ja = pjk.tile([P, N], F32, tag="junka", name="junka")
        nc.scalar.activation(ja[:, 0:32], zt[:], ACTF.Square,
                             accum_out=qpack[:, t:t + 1])
    mu = pst.tile([P, 12], F32, tag="mu", name="mu")
    nc.vector.tensor_scalar(mu[:], spack[:], 1.0 / 32, 0.0, op0=ALU.mult)
    m2 = pst.tile([P, 12], F32, tag="m2", name="m2")
    nc.vector.tensor_tensor(m2[:], mu[:], mu[:], op=ALU.mult)
    var = pst.tile([P, 12], F32, tag="var", name="var")
    nc.vector.scalar_tensor_tensor(var[:], qpack[:], 1.0 / 32, m2[:],
                                   op0=ALU.mult, op1=ALU.subtract)
    eps_col = pst.tile([P, 1], F32, tag="eps_col", name="eps_col")
    nc.any.memset(eps_col[:], 1e-5)
    sd = pst.tile([P, 12], F32, tag="sd", name="sd")
    nc.scalar.activation(sd[:], var[:], ACTF.Sqrt, bias=eps_col[:])
    inv = pst.tile([P, 12], F32, tag="inv", name="inv")
    nc.vector.reciprocal(inv[:], sd[:])
    gam = wslice('bn_scale')
    bet = wslice('bn_shift')
    sc = pst.tile([P, 12], F32, tag="sc", name="sc")
    nc.vector.tensor_tensor(sc[:], inv[:], gam, op=ALU.mult)
    bi = pst.tile([P, 12], F32, tag="bi", name="bi")
    nc.vector.tensor_tensor(bi[:], mu[:], sc[:], op=ALU.mult)
    nc.vector.tensor_tensor(bi[:], bet, bi[:], op=ALU.subtract)
    zcur = []
    for t in range(12):
        zc = pst.tile([P, 32], F16, tag=f"zc{t}", name=f"zc{t}")
        nc.scalar.activation(zc[:], zT[t][:], ACTF.Identity,
                             bias=bi[:, t:t + 1], scale=sc[:, t:t + 1])
        zcur.append(zc[:])

    linBc = wslice('linBcol')
    for li in range(LIN_D):
        wt = wts[li]
        psA = seq_ps(P, 32)
        psB = seq_ps(64, 32)
        for k in range(12):
            wa = wt[:, HSH * k:HSH * k + 128]
            wb = wt[:, HSH * k + 128:HSH * (k + 1)]
            nc.tensor.matmul(psA[:], wa, zcur[k], start=(k == 0),
                             stop=(k == 11))
            nc.tensor.matmul(psB[:], wb, zcur[k], start=(k == 0),
                             stop=(k == 11))
        zslA = pwork.tile([P, 32], F16, tag="zslA", name="zslA")
        zslB = pwork.tile([64, 32], F16, tag="zslB", name="zslB")
        nc.scalar.activation(zslA[:], psA[:], ACTF.Lrelu,
                             bias=linBc[:, 2 * li:2 * li + 1], alpha=0.01)
        nc.scalar.activation(zslB[:], psB[0:64, :], ACTF.Lrelu,
                             bias=linBc[0:64, 2 * li + 1:2 * li + 2],
                             alpha=0.01)
        if li < LIN_D - 1:
            dma(io['zsl'][li][0:128, :], zslA[:])
            dma(io['zsl'][li][128:192, :], zslB[:])
            nc.gpsimd.collective_compute(
                "AllGather", ALU.bypass, replica_groups=[cores],
                ins=[io['zsl'][li][:]], outs=[io['zfull'][li][:]])
            zcat = pst.tile([P, 12 * 32], F16, tag=f"zcat{li}", name=f"zcat{li}")
            dma(zcat[:].rearrange("p (k b) -> p k b", k=12),
                io['zfull'][li].rearrange("(k p) b -> p k b", k=12))
            zcur = [zcat[:, 32 * t:32 * (t + 1)] for t in range(12)]
    # final layer: each core holds its 192-row slice of z4; compute the
    # local contribution to out = z4 @ out_W, AllGather the partials, sum.
    outWc = wslice('outWc')
    outWh = pwork.tile([P, 2], F16, tag="outWh", name="outWh")
    nc.vector.tensor_copy(outWh[:], outWc)
    outb_sb = wslice('outb', 1)
    ps_out = seq_ps(1, 32)
    nc.tensor.matmul(ps_out[:], outWh[:, 0:1], zslA[:],
                     start=True, stop=False)
    nc.tensor.matmul(ps_out[:], outWh[0:64, 1:2], zslB[:],
                     start=False, stop=True)
    o32 = pwork.tile([1, 32], F32, tag="o32", name="o32")
    nc.scalar.copy(o32[:], ps_out[:])
    dma(io['opart'][:], o32[:])
    nc.gpsimd.collective_compute(
        "AllGather", ALU.bypass, replica_groups=[cores],
        ins=[io['opart'][:]], outs=[io['opart8'][:]])
    op8 = pwork.tile([8, 32], F32, tag="op8", name="op8", padded_shape=[128, 32])
    dma(op8[:], io['opart8'][:])
    osum_ps = seq_ps(1, 32)
    nc.tensor.matmul(osum_ps[:], allones[0:8, 0:1], op8[0:8, :],
                     start=True, stop=True)
    o32f = pwork.tile([1, 32], F32, tag="o32f", name="o32f")
    nc.scalar.activation(o32f[:], osum_ps[:], ACTF.Identity, bias=outb_sb)
    dma(io['out32'][:], o32f[:])


def build_nc(skip_head=False):
    nc = bacc.Bacc("TRN2", target_bir_lowering=False, debug=False,
                   num_devices=NCORES)
    io = {}

    def inp(name, shape):
        io[name] = nc.dram_tensor(name, list(shape), F32, kind="ExternalInput").ap()

    inp('xpack', (GPC, 128, 2 * N + 24))
    inp('wpack', (128, WPACK_COLS))
    io['linW'] = nc.dram_tensor("linW", [LIN_D, 12, 128, HSH], F16,
                                kind="ExternalInput").ap()

    io['z0loc'] = nc.dram_tensor(
        "z0loc", [GPC, DIM2], F32,
        kind="ExternalOutput" if skip_head else "Internal").ap()
    io['out32'] = nc.dram_tensor("out32", [1, 32], F32,
                                 kind="ExternalOutput").ap()
    if not skip_head:
        io['z0all'] = nc.dram_tensor("z0all", [B, DIM2], F32,
                                     addr_space="Shared").ap()
        io['zsl'] = [nc.dram_tensor(f"zsl{li}", [HSH, 32], F16).ap()
                     for li in range(LIN_D)]
        io['zfull'] = [nc.dram_tensor(f"zfull{li}", [DIM2, 32], F16,
                                      addr_space="Shared").ap()
                       for li in range(LIN_D - 1)]
        io['opart'] = nc.dram_tensor("opart", [1, 32], F32).ap()
        io['opart8'] = nc.dram_tensor("opart8", [8, 32], F32,
                                      addr_space="Shared").ap()

    with tile.TileContext(nc) as tc:
        core_program(tc, io, skip_head=skip_head)
    nc.compile()
    return nc


_CACHED = {}
_DEBUG_IO = None


def kernel(**inputs) -> np.ndarray:
    from concourse.bass_utils import run_bass_kernel_spmd
    if 'nc' not in _CACHED:
        _CACHED['nc'] = build_nc()
    nc = _CACHED['nc']
    in_maps = []
    for c in range(NCORES):
        d = prep_host(inputs, c)
        in_maps.append({k: np.ascontiguousarray(v) for k, v in d.items()})
    res = run_bass_kernel_spmd(nc, in_maps, core_ids=list(range(NCORES)),
                               trace=bool(os.environ.get("KBENCH_TRACE")))
    _CACHED['last'] = res
    return res.results[0]['out32'].reshape(-1).astype(np.float32)


if __name__ == "__main__":
    data = dict(np.load('/root/problem/inputs.npz'))
    out = kernel(**data)
    print("kernel out:", out[:5])



# revision 64
# speedup vs baseline: 1.0986x; 1.0680x over previous
"""Trainium2 Bass kernel for nn_EnsembleNet3 (gnn_message_passing).

Self-contained: takes full inputs (as produced by setup_inputs()), shards the
B=32 graph dim over 8 NeuronCores (4 graphs/core), runs the graph stack fully
on-device, and computes the [B,1536] head sharded over output columns with
AllGathers between layers.

Per graph (N=512 nodes):
- kNN-100 for TAGConv: per-row threshold via count-secant iteration on
  Q[i,j] = 2*G[i,j] - n_j (same per-row order as -dist; Q row max is self),
  finished by an exact masked max-8 endgame; adjacency applied as a dense 0/1
  mask matmul on PE with host-folded hop weights:
  out = h@W~0 + (Mh)@W~1 + (M^2 h)@W~2,  M = mask incl self.
- EdgeConv k=3: top-3 indices via max8+max_index on Q; gather done as an
  exact one-hot selection matmul on PE: Sel_t[j,i] = (idx_l(i) == 128t+j)
  built by DVE compares from a PE-replicated index row, then
  gathered = g^T~t . Sel_t accumulated over tiles in PSUM. MLPs decomposed
  as a_i + g_j so only g is gathered; max-aggregation commutes with
  leaky_relu. leaky_relu computed as max(x, 0.01x) in one fused DVE op
  (with free mean-pool accumulation) or via ACT Lrelu with bias folding.
"""
import os
from contextlib import ExitStack

import numpy as np

import concourse.bass as bass
import concourse.bacc as bacc
import concourse.tile as tile
from concourse import mybir
from concourse._compat import with_exitstack

F32 = mybir.dt.float32
F32R = mybir.dt.float32r
F16 = mybir.dt.float16
U16 = mybir.dt.uint16
U32 = mybir.dt.uint32
U8 = mybir.dt.uint8
ALU = mybir.AluOpType
ACTF = mybir.ActivationFunctionType
AXX = mybir.AxisListType.X

B, N, F, W = 32, 512, 6, 128
NT = N // 128
GPC = 4
NCORES = 8
K101 = 101
SEL_ITERS = 8
SEL_TARGET = float(K101 + 4)
U_LO, U_HI = -16.0, 16.0
DIM2 = 1536
HSH = DIM2 // NCORES
LIN_D = 5
DVE_COLS = 7    # selection count passes: cols < DVE_COLS on DVE, rest on ACT


def _fold_tag(Wk, b):
    W0, W1, W2 = Wk[0], Wk[1], Wk[2]
    c1, c2 = 1.0 / 100.0, 1.0 / 10000.0
    return (
        (W0 - W1 * c1 + W2 * c2).astype(np.float32),
        (W1 * c1 - 2.0 * W2 * c2).astype(np.float32),
        (W2 * c2).astype(np.float32),
        b.astype(np.float32),
    )


def prep_host(inputs, core):
    inp = {k: np.asarray(v) for k, v in inputs.items()}
    x = inp['x'].astype(np.float32).reshape(B, N, F)
    xs = x[GPC * core:GPC * (core + 1)]
    f32 = np.float32

    # --- per-graph input pack [128, 1048]: xgT | xgR | xnm ---
    xt = xs.transpose(0, 2, 1)
    xpack = np.zeros((GPC, 128, 2 * N + 24), f32)
    xpack[:, 0:F, 0:N] = xt
    xpack[:, F:2 * F, 0:N] = 1.0
    xpack[:, 0:F, N:2 * N] = 2.0 * xt
    xpack[:, F:2 * F, N:2 * N] = -(xt * xt)
    xpack[:, 12, N:2 * N] = 1.0
    for t in range(NT):
        xpack[:, :, 2 * N + F * t:2 * N + F * (t + 1)] = xs[:, 128 * t:128 * (t + 1), :]

    # --- const pack [128, cols] ---
    cols = {}
    blobs = []
    off = 0

    def put(name, arr2d):
        nonlocal off
        a = np.asarray(arr2d, f32)
        pad = np.zeros((128, a.shape[1]), f32)
        pad[:a.shape[0]] = a
        cols[name] = (off, a.shape[1])
        blobs.append(pad)
        off += a.shape[1]

    put('eye', np.eye(128, dtype=f32))
    put('iota8', np.broadcast_to(np.arange(8, dtype=f32), (128, 8)))
    put('iotaP', np.arange(128, dtype=f32).reshape(128, 1))
    rsel = np.zeros((3, 384), f32)
    for l in range(3):
        rsel[l, 128 * l:128 * (l + 1)] = 1.0
    put('rsel3', rsel)
    for li, (Wk, bk) in enumerate(
            [(inp['tag1_W'], inp['tag1_b']),
             (inp['tag_W'][0], inp['tag_b'][0]),
             (inp['tag_W'][1], inp['tag_b'][1])]):
        w0, w1, w2, bb = _fold_tag(Wk, bk)
        put(f'tagw{li}', np.concatenate([w0, w1, w2], axis=1))
        put(f'tagb{li}', bb.reshape(128, 1))
    W1 = inp['p1_W1'].astype(f32)
    put('ec1_a', W1[:F] - W1[F:])
    put('ec1_g', W1[F:])
    put('ec1_b1', inp['p1_b1'].astype(f32).reshape(128, 1))
    put('ec1_w2', inp['p1_W2'].astype(f32))
    put('ec1_b2', inp['p1_b2'].astype(f32).reshape(128, 1))
    for f in range(2):
        Wf = inp['pf_W'][f].astype(f32)
        put(f'ec{f+2}_a', Wf[:W] - Wf[W:])
        put(f'ec{f+2}_g', Wf[W:])
        put(f'ec{f+2}_b', inp['pf_b'][f].astype(f32).reshape(128, 1))
    put('bn_scale', inp['bn_gamma'].astype(f32).reshape(12, 128).T)
    put('bn_shift', inp['bn_beta'].astype(f32).reshape(12, 128).T)
    put('outW', inp['out_W'].astype(f32).reshape(12, 128).T)
    put('outb', np.full((1, 1), float(inp['out_b'][0]), f32))
    sl = slice(HSH * core, HSH * (core + 1))
    put('linB', inp['lin_b'].astype(f32)[:, sl].reshape(1, LIN_D * HSH))
    lbc = np.zeros((128, 2 * LIN_D), f32)
    for li in range(LIN_D):
        lb = inp['lin_b'].astype(f32)[li, sl]
        lbc[0:128, 2 * li] = lb[0:128]
        lbc[0:64, 2 * li + 1] = lb[128:192]
    put('linBcol', lbc)
    owc = np.zeros((128, 2), f32)
    ow = inp['out_W'].astype(f32).reshape(-1)[sl]
    owc[0:128, 0] = ow[0:128]
    owc[0:64, 1] = ow[128:192]
    put('outWc', owc)
    wpack = np.concatenate(blobs, axis=1)
    assert wpack.shape[1] == WPACK_COLS, (wpack.shape, off)
    assert all(cols[k] == WOFF[k] for k in cols), "WOFF mismatch"

    linW = inp['lin_W'].astype(np.float16)
    d = {
        'xpack': np.ascontiguousarray(xpack),
        'wpack': np.ascontiguousarray(wpack),
        'linW': np.ascontiguousarray(linW[:, :, sl].reshape(LIN_D, 12, 128, HSH)),
    }
    return d


def _woff_table():
    off = 0
    tab = {}
    def put(name, w):
        nonlocal off
        tab[name] = (off, w)
        off += w
    put('eye', 128); put('iota8', 8); put('iotaP', 1); put('rsel3', 384)
    for li in range(3):
        put(f'tagw{li}', 384); put(f'tagb{li}', 1)
    put('ec1_a', 128); put('ec1_g', 128); put('ec1_b1', 1)
    put('ec1_w2', 128); put('ec1_b2', 1)
    for f in range(2):
        put(f'ec{f+2}_a', 128); put(f'ec{f+2}_g', 128); put(f'ec{f+2}_b', 1)
    put('bn_scale', 12); put('bn_shift', 12)
    put('outW', 12); put('outb', 1)
    put('linB', LIN_D * HSH); put('linBcol', 2 * LIN_D); put('outWc', 2)
    return tab, off


WOFF, WPACK_COLS = _woff_table()


@with_exitstack
def core_program(ctx: ExitStack, tc: tile.TileContext, io: dict, skip_head=False):
    nc = tc.nc
    P = 128

    const = ctx.enter_context(tc.tile_pool(name="const", bufs=1))
    pq = ctx.enter_context(tc.tile_pool(name="pq", bufs=16))
    pq2 = ctx.enter_context(tc.tile_pool(name="pq2", bufs=1))
    pmask = ctx.enter_context(tc.tile_pool(name="pmask", bufs=8))
    pwork = ctx.enter_context(tc.tile_pool(name="pwork", bufs=1))
    psel = ctx.enter_context(tc.tile_pool(name="psel", bufs=8))
    phid = ctx.enter_context(tc.tile_pool(name="phid", bufs=2))
    phT = ctx.enter_context(tc.tile_pool(name="phT", bufs=4))
    pyT = ctx.enter_context(tc.tile_pool(name="pyT", bufs=3))
    phn = ctx.enter_context(tc.tile_pool(name="phn", bufs=8))
    pst = ctx.enter_context(tc.tile_pool(name="pst", bufs=1))
    phw = ctx.enter_context(tc.tile_pool(name="phw", bufs=1))
    psq = ctx.enter_context(tc.tile_pool(name="psq", bufs=4, space="PSUM"))
    pss = ctx.enter_context(tc.tile_pool(name="pss", bufs=2, space="PSUM"))

    def quad_ps(pp=P, nn=N):
        return psq.tile([pp, nn], F32, tag="quad", name="quad")

    def seq_ps(pp, nn):
        return pss.tile([pp, nn], F32, tag="seq", name="seq")

    def dma(dst, src):
        nc.sync.dma_start(dst, src)

    # All value matmuls are exact fp32: the head BatchNorm divides pooled
    # features by ~3e-3 (tiny batch variance), amplifying any value-path
    # rounding ~300x; f32r's 2.4e-4 rounding would blow the 2e-2 gate.
    # f32r is used ONLY where operands are exact small ints / 0-1 (rep).
    def mmf(out, lhsT, rhs, start, stop):
        nc.tensor.matmul(out, lhsT, rhs, start=start, stop=stop)

    def mmr(out, lhsT, rhs, start, stop):
        # f32r: 1 cyc/row vs fp32's 4. Only on paths that tolerate the
        # ~2.4e-4 operand rounding (gather payloads, 2nd-hop aggregates);
        # never on x/y/weights that feed rankings or the pools directly.
        nc.tensor.matmul(out, lhsT.bitcast(F32R), rhs.bitcast(F32R),
                         start=start, stop=stop)

    # ---- constants: one packed DMA ----
    # DMA'd with f32r dtype so the weights count as f32r-rounded producers
    # for the walrus verifier (bytes unchanged; PE truncates on read).
    wp = const.tile([P, WPACK_COLS], F32, tag="wpack", name="wpack")
    dma(wp[:], io['wpack'][:])

    def wslice(name, rows=128):
        o, w = WOFF[name]
        return wp[0:rows, o:o + w]

    eye = wslice('eye')
    iota8 = wslice('iota8')
    iotaP = wslice('iotaP')
    rsel3 = wslice('rsel3', 3)
    ones_col = const.tile([P, 1], F32)
    nc.any.memset(ones_col[:], 1.0)
    ones32 = const.tile([1, 32], F32, padded_shape=[128, 32])
    nc.any.memset(ones32[:], 1.0)
    allones = const.tile([P, P], F32)
    nc.any.memset(allones[:], 1.0)

    tagw, tagb = [], []
    for li in range(3):
        fin = F if li == 0 else W
        wt = wslice(f'tagw{li}', fin)
        tagw.append([wt[:, 128 * k:128 * (k + 1)] for k in range(3)])
        tagb.append(wslice(f'tagb{li}'))

    rs_o, rs_w = WOFF['rsel3']
    rsel3r_t = const.tile([P, rs_w], F32, tag="r_rsel3", name="r_rsel3")
    nc.vector.tensor_copy(rsel3r_t[:].bitcast(F32R), wp[0:P, rs_o:rs_o + rs_w])
    rsel3r = rsel3r_t[0:3, :]

    ec1_a = wslice('ec1_a', F)
    ec1_g = wslice('ec1_g', F)
    ec1_w2 = wslice('ec1_w2')
    ec1_b1 = wslice('ec1_b1')
    ec1_b2 = wslice('ec1_b2')
    ecf_a = [wslice('ec2_a'), wslice('ec3_a')]
    ecf_g = [wslice('ec2_g'), wslice('ec3_g')]
    ecf_b = [wslice('ec2_b'), wslice('ec3_b')]

    # ---- inputs per graph: one packed DMA each ----
    xgT, xgR, xnm = [], [], []
    for g in range(GPC):
        xp = pst.tile([P, 2 * N + 24], F32, tag=f"xpack{g}", name=f"xpack{g}")
        dma(xp[:], io['xpack'][g])
        xgT.append(xp[:, 0:N])
        xgR.append(xp[:, N:2 * N])
        xnm.append([xp[:, 2 * N + F * t:2 * N + F * (t + 1)] for t in range(NT)])

    # ---- prefetch head weights early (fp16): streams during graph phase ----
    wts = []
    for li in range(LIN_D):
        wt = phw.tile([P, 12 * HSH], F16, tag=f"linW{li}", name=f"linW{li}")
        dma(wt[:].rearrange("a (k b) -> a k b", k=12),
            io['linW'][li].rearrange("k a b -> a k b"))
        wts.append(wt)

    # ---- Q = 2G - n_row via augmented matmul (K=12), n_col from x_nm ----
    Q = [[None] * NT for _ in range(GPC)]
    for g in range(GPC):
        gps = [quad_ps() for _ in range(NT)]
        for t in range(NT):
            nc.tensor.matmul(gps[t][:], xgT[g][0:12, 128 * t:128 * (t + 1)],
                             xgR[g][0:12, 0:N], start=True, stop=True)
        for t in range(NT):
            qt = pq.tile([P, N], F32, tag="Q", name="Q")
            nc.scalar.copy(qt[:], gps[t][:])
            Q[g][t] = qt

    # ---- per-graph count-secant selection (pipelines across graphs) ----
    # DVE cols [0, SEL_DVE), ACT cols [SEL_DVE, NT) per graph per iteration.
    SEL_DVE = 2
    pjk = ctx.enter_context(tc.tile_pool(name="pjk", bufs=2))

    def sel_graph(g):
        """Exact 101st-largest threshold per Q row -> ustar_g [P, NT]."""
        def t4(nm, init=None):
            tt = pst.tile([P, NT], F32, tag=f"{nm}{g}", name=f"{nm}{g}")
            if init is not None:
                nc.any.memset(tt[:], init)
            return tt
        st_u = t4('su', U_HI + (U_LO - U_HI) * (SEL_TARGET / N))
        st_ul = t4('sul', U_HI)
        st_uh = t4('suh', U_LO)
        st_cl = t4('scl', 0.0)
        st_ch = t4('sch', float(N))
        cnt = t4('scn')
        tmp_a = t4('sta')
        tmp_b = t4('stb')
        tmp_m = pst.tile([P, NT], U8, tag=f"stm{g}", name=f"stm{g}")
        for it in range(SEL_ITERS):
            for t in range(NT):
                ucol = st_u[:, t:t + 1]
                ccol = cnt[:, t:t + 1]
                if t < SEL_DVE:
                    jd = pjk.tile([P, N], F32, tag="junkd", name="junkd")
                    nc.vector.tensor_scalar(
                        jd[:], Q[g][t][:], ucol, 0.0,
                        op0=ALU.is_ge, op1=ALU.add, accum_out=ccol)
                else:
                    ja = pjk.tile([P, N], F32, tag="junka", name="junka")
                    nc.scalar.activation(
                        ja[:], Q[g][t][:], ACTF.Sign,
                        bias=ucol, scale=-1.0, accum_out=ccol)
            # ACT cols report sum(sign): c = 256 - s/2
            nc.vector.tensor_scalar(
                cnt[:, SEL_DVE:NT], cnt[:, SEL_DVE:NT], -0.5, 256.0,
                op0=ALU.mult, op1=ALU.add)
            nc.vector.tensor_scalar(
                tmp_m[:], cnt[:], float(K101) - 0.5, 0.0, op0=ALU.is_ge)
            nc.vector.copy_predicated(st_uh[:], tmp_m[:], st_u[:])
            nc.vector.copy_predicated(st_ch[:], tmp_m[:], cnt[:])
            nc.vector.tensor_scalar(
                tmp_m[:], cnt[:], float(K101) - 0.5, 0.0, op0=ALU.is_lt)
            nc.vector.copy_predicated(st_ul[:], tmp_m[:], st_u[:])
            nc.vector.copy_predicated(st_cl[:], tmp_m[:], cnt[:])
            if it == SEL_ITERS - 1:
                break
            nc.vector.tensor_tensor(tmp_a[:], st_ch[:], st_cl[:],
                                    op=ALU.subtract)
            nc.vector.reciprocal(tmp_a[:], tmp_a[:])
            nc.vector.scalar_tensor_tensor(
                tmp_b[:], st_ch[:], -SEL_TARGET, tmp_a[:],
                op0=ALU.add, op1=ALU.mult)
            nc.vector.tensor_scalar(
                tmp_b[:], tmp_b[:], 0.05, 0.95, op0=ALU.max, op1=ALU.min)
            nc.vector.tensor_tensor(tmp_a[:], st_ul[:], st_uh[:],
                                    op=ALU.subtract)
            nc.vector.tensor_tensor(tmp_a[:], tmp_a[:], tmp_b[:], op=ALU.mult)
            nc.vector.tensor_tensor(st_u[:], st_uh[:], tmp_a[:], op=ALU.add)
        # endgame: exact 101st-largest via masked top-8
        ustar_g = t4('sus')
        pos_g = t4('sps')
        nc.vector.tensor_scalar(pos_g[:], st_ch[:], -float(K101), 0.0,
                                op0=ALU.add)
        for t in range(NT):
            zm = pwork.tile([P, N], F32, tag="zm", name="zm")
            nc.vector.tensor_scalar(
                zm[:], Q[g][t][:], st_uh[:, t:t + 1], -1e30,
                op0=ALU.is_lt, op1=ALU.mult)
            nc.vector.tensor_tensor(zm[:], zm[:], Q[g][t][:], op=ALU.subtract)
            m8 = pwork.tile([P, 8], F32, tag="m8e", name="m8e")
            nc.vector.max(m8[:], zm[:])
            msk8 = pwork.tile([P, 8], F32, tag="msk8", name="msk8")
            nc.vector.tensor_tensor(
                msk8[:], iota8,
                pos_g[:, t:t + 1].broadcast_to([P, 8]), op=ALU.is_equal)
            j8 = pwork.tile([P, 8], F32, tag="j8", name="j8")
            nc.vector.scalar_tensor_tensor(
                j8[:], m8[:], -1.0, msk8[:], op0=ALU.mult, op1=ALU.mult,
                accum_out=ustar_g[:, t:t + 1])
        return ustar_g

    lrelu_op = dict(op0=ALU.mult, op1=ALU.max)

    def lrelu_into(dst, src, accum=None):
        nc.vector.scalar_tensor_tensor(dst, src, 0.01, src, accum_out=accum,
                                       **lrelu_op)

    def topk3(Qt, lo=1):
        """top-3 neighbor indices per node -> ts3s [3, N] fp32.

        Takes ranks lo..lo+2. lo=1 skips rank 0 (self, when Q row max is
        self); lo=0 is used when the diagonal is already masked out.
        """
        ts3 = seq_ps(3, N)
        for t in range(NT):
            m8 = pwork.tile([P, 8], F32, tag="m8g", name="m8g")
            nc.vector.max(m8[:], Qt[t][:])
            i8 = pwork.tile([P, 8], U32, tag="i8g", name="i8g")
            nc.vector.max_index(i8[:], m8[:], Qt[t][:])
            i8f = pwork.tile([P, 8], F32, tag="i8f", name="i8f")
            nc.vector.tensor_copy(i8f[:], i8[:])
            nc.tensor.transpose(ts3[0:3, 128 * t:128 * (t + 1)],
                                i8f[:, lo:lo + 3], eye)
        ts3s = pwork.tile([3, N], F32, tag="ts3s", name="ts3s", padded_shape=[128, N])
        nc.scalar.copy(ts3s[:].bitcast(F32R), ts3[0:3, :])
        return ts3s

    def sel_gather(ts3s, l, lhsTs, ps, close=True):
        """ps[f, i] (+)= payload[f, idx_l(i)] via one-hot Sel matmuls.

        Sel_t[j, i] = (idx_l(i) == 128t + j), built by DVE compare from a
        PE-replicated index row; lhsTs[t] = node-major payload [j, f].
        """
        rep_ps = seq_ps(P, N)
        nc.tensor.matmul(rep_ps[:],
                         rsel3r[0:3, 128 * l:128 * (l + 1)].bitcast(F32R),
                         ts3s[0:3, 0:N].bitcast(F32R), start=True, stop=True)
        for t in range(NT):
            sel = psel.tile([P, N], F32, tag="sel", name="sel")
            nc.vector.tensor_scalar(
                sel[:], rep_ps[:], iotaP, float(128 * t),
                op0=ALU.subtract, op1=ALU.is_equal)
            mmf(ps[:], lhsTs[t][:], sel[:], start=(t == 0),
                stop=(close and t == NT - 1))

    zpack = [pst.tile([P, 12], F32, tag=f"zpack{g}", name=f"zpack{g}") for g in range(GPC)]
    dbg = globals().get('_DEBUG_IO')

    for g in range(GPC):
        ustar_g = sel_graph(g)
        # ---- maskT: M[i,j] = (Q[i,j] >= u*_i) on DVE, then PE transposes ----
        Mrows = []
        for t in range(NT):
            mrow = pwork.tile([P, N], F32, tag="Mrow", name="Mrow")
            nc.vector.tensor_scalar(
                mrow[:], Q[g][t][:], ustar_g[:, t:t + 1], 0.0,
                op0=ALU.is_ge)
            Mrows.append(mrow)
        maskT = []
        for jc in range(NT):
            mps = quad_ps()
            for it in range(NT):
                nc.tensor.transpose(mps[0:P, 128 * it:128 * (it + 1)],
                                    Mrows[it][:, 128 * jc:128 * (jc + 1)], eye)
            mt = pmask.tile([P, N], F32, tag="maskT", name="maskT")
            nc.scalar.copy(mt[:], mps[:])
            maskT.append(mt)

        # ---- TAG ----
        hT = xgT[g][0:F, 0:N]
        hnm = xnm[g]
        for li in range(3):
            fin = F if li == 0 else W
            u1n_ps = [quad_ps(P, fin) for _ in range(NT)]
            for ic in range(NT):
                for jc in range(NT):
                    mmf(u1n_ps[ic][0:P, 0:fin],
                        maskT[jc][:, 128 * ic:128 * (ic + 1)],
                        hnm[jc][:], start=(jc == 0), stop=(jc == NT - 1))
            u1n = []
            for ic in range(NT):
                uu = phn.tile([P, fin], F32, tag="u1n", name="u1n")
                nc.scalar.copy(uu[:], u1n_ps[ic][0:P, 0:fin])
                u1n.append(uu)
            u1T_ps = seq_ps(fin, N)
            for ic in range(NT):
                nc.tensor.transpose(u1T_ps[0:fin, 128 * ic:128 * (ic + 1)],
                                    u1n[ic][:], eye)
            u1T = pwork.tile([fin, N], F32, tag="u1T", name="u1T", padded_shape=[128, N])
            nc.scalar.copy(u1T[:], u1T_ps[0:fin, :])
            u2T_ps = seq_ps(fin, N)
            for jc in range(NT):
                mmf(u2T_ps[0:fin, :], u1n[jc][:], maskT[jc][:],
                    start=(jc == 0), stop=(jc == NT - 1))
            u2T = pwork.tile([fin, N], F32, tag="u2T", name="u2T", padded_shape=[128, N])
            nc.scalar.copy(u2T[:], u2T_ps[0:fin, :])
            oT_ps = seq_ps(P, N)
            nc.tensor.matmul(oT_ps[:], tagw[li][0], hT[:], start=True, stop=False)
            nc.tensor.matmul(oT_ps[:], tagw[li][1], u1T[:], start=False, stop=False)
            nc.tensor.matmul(oT_ps[:], tagw[li][2], u2T[:], start=False, stop=True)
            sT = pwork.tile([P, N], F32, tag="sT", name="sT")
            nc.scalar.activation(sT[:], oT_ps[:], ACTF.Identity, bias=tagb[li])
            hT_new = phT.tile([P, N], F32, tag="hT", name="hT")
            lrelu_into(hT_new[:], sT[:],
                       accum=zpack[g][:, 2 * li:2 * li + 1])
            nc.vector.tensor_reduce(zpack[g][:, 2 * li + 1:2 * li + 2], hT_new[:],
                                    axis=AXX, op=ALU.max)
            if g == 0 and dbg is not None and 'dbgH' in dbg:
                dma(dbg['dbgH'][li], hT_new[:])
            hT = hT_new
            if li < 2:
                hnm = []
                for t in range(NT):
                    hps = quad_ps(P, P)
                    nc.tensor.transpose(hps[0:P, 0:P], hT[:, 128 * t:128 * (t + 1)],
                                        eye)
                    hh = phn.tile([P, P], F32, tag="hnm", name="hnm")
                    nc.scalar.copy(hh[:], hps[0:P, 0:P])
                    hnm.append(hh)

        # ---- EC1 ----
        g1T = []
        for t in range(NT):
            gps = quad_ps(P, P)
            nc.tensor.matmul(gps[0:P, 0:P], xgT[g][0:F, 128 * t:128 * (t + 1)],
                             ec1_g, start=True, stop=True)
            gt = phn.tile([P, P], F32, tag="gT", name="gT")
            nc.scalar.copy(gt[:], gps[0:P, 0:P])
            g1T.append(gt)
        ts1 = topk3(Q[g])
        mx = pwork.tile([P, N], F32, tag="mx", name="mx")
        for l in range(3):
            hid_ps = quad_ps()
            sel_gather(ts1, l, g1T, hid_ps, close=False)
            mmf(hid_ps[:], ec1_a, xgT[g][0:F, 0:N], start=False, stop=True)
            hid = phid.tile([P, N], F32, tag="hid", name="hid")
            nc.scalar.activation(hid[:], hid_ps[:], ACTF.Lrelu,
                                 bias=ec1_b1, alpha=0.01)
            m_ps = seq_ps(P, N)
            mmf(m_ps[:], ec1_w2, hid[:], start=True, stop=True)
            if l == 0:
                nc.vector.tensor_scalar(mx[:], m_ps[:], ec1_b2, None,
                                        op0=ALU.add)
            else:
                nc.vector.scalar_tensor_tensor(mx[:], m_ps[:], ec1_b2, mx[:],
                                               op0=ALU.add, op1=ALU.max)
        yT = pyT.tile([P, N], F32, tag="yT", name="yT")
        lrelu_into(yT[:], mx[:], accum=zpack[g][:, 6:7])
        if g == 0 and dbg is not None and 'dbgY' in dbg:
            dma(dbg['dbgY'][0], yT[:])
        nc.vector.tensor_reduce(zpack[g][:, 9:10], yT[:], axis=AXX, op=ALU.max)

        # ---- EC2 / EC3 ----
        for f in range(2):
            y2 = pwork.tile([P, N], F32, tag="y2", name="y2")
            nc.vector.tensor_scalar(y2[:], yT[:], 2.0, 0.0, op0=ALU.mult)
            nysq = pwork.tile([P, N], F32, tag="nysq", name="nysq")
            nc.vector.scalar_tensor_tensor(nysq[:], yT[:], -1.0, yT[:],
                                           op0=ALU.mult, op1=ALU.mult)
            gy_ps = [quad_ps() for _ in range(NT)]
            for t in range(NT):
                nc.tensor.matmul(gy_ps[t][:], y2[:, 128 * t:128 * (t + 1)],
                                 yT[:], start=True, stop=False)
                nc.tensor.matmul(gy_ps[t][:], allones[:], nysq[:],
                                 start=False, stop=True)
            Q2 = []
            for t in range(NT):
                q2 = pq2.tile([P, N], F32, tag=f"Q2{t}", name=f"Q2{t}")
                nc.scalar.copy(q2[:], gy_ps[t][:])
                nc.vector.scalar_tensor_tensor(
                    q2[:, 128 * t:128 * (t + 1)], eye, -1e30,
                    q2[:, 128 * t:128 * (t + 1)], op0=ALU.mult, op1=ALU.add)
                Q2.append(q2)
            gfT = []
            for t in range(NT):
                gps = quad_ps(P, P)
                nc.tensor.matmul(gps[0:P, 0:P], yT[:, 128 * t:128 * (t + 1)],
                                 ecf_g[f], start=True, stop=True)
                gt = phn.tile([P, P], F32, tag="gT", name="gT")
                nc.scalar.copy(gt[:], gps[0:P, 0:P])
                gfT.append(gt)
            af_ps = seq_ps(P, N)
            nc.tensor.matmul(af_ps[:], ecf_a[f], yT[:], start=True, stop=True)

            ts2 = topk3(Q2, lo=0)
            mx2 = pwork.tile([P, N], F32, tag="mx2", name="mx2")
            for l in range(3):
                gps = quad_ps()
                sel_gather(ts2, l, gfT, gps)
                if l == 0:
                    nc.vector.tensor_copy(mx2[:], gps[:])
                else:
                    nc.vector.tensor_tensor(mx2[:], mx2[:], gps[:], op=ALU.max)
            nc.vector.scalar_tensor_tensor(mx2[:], af_ps[:], ecf_b[f], mx2[:],
                                           op0=ALU.add, op1=ALU.add)
            yT_new = pyT.tile([P, N], F32, tag="yT", name="yT")
            lrelu_into(yT_new[:], mx2[:], accum=zpack[g][:, 7 + f:8 + f])
            if g == 0 and dbg is not None and 'dbgY' in dbg:
                dma(dbg['dbgY'][1 + f], yT_new[:])
            nc.vector.tensor_reduce(zpack[g][:, 10 + f:11 + f], yT_new[:],
                                    axis=AXX, op=ALU.max)
            yT = yT_new

        # ---- pools -> z0 row ----
        means_a = zpack[g][:].rearrange("p (a b) -> p a b", a=6, b=2)[:, 0:3, 0:1]
        nc.vector.tensor_scalar(means_a, means_a, 1.0 / N, 0.0, op0=ALU.mult)
        nc.vector.tensor_scalar(zpack[g][:, 6:9], zpack[g][:, 6:9], 1.0 / N, 0.0,
                                op0=ALU.mult)
        zr_ps = seq_ps(12, P)
        nc.tensor.transpose(zr_ps[0:12, 0:P], zpack[g][:], eye)
        zrow = pwork.tile([12, P], F32, tag="zrow", name="zrow")
        nc.scalar.copy(zrow[:], zr_ps[0:12, 0:P])
        dma(io['z0loc'][g].rearrange("(a b) -> a b", a=12), zrow[:])

    if skip_head:
        return

    # ---- head ----
    cores = list(range(NCORES))
    nc.gpsimd.collective_compute(
        "AllGather", ALU.bypass, replica_groups=[cores],
        ins=[io['z0loc'][:]], outs=[io['z0all'][:]])
    z0s = pst.tile([32, DIM2], F32, tag="z0s", name="z0s", padded_shape=[128, DIM2])
    dma(z0s[:], io['z0all'][:])
    tc.strict_bb_all_engine_barrier()
    zT = []
    spack = pst.tile([P, 12], F32, tag="spack", name="spack")
    qpack = pst.tile([P, 12], F32, tag="qpack", name="qpack")
    for t in range(12):
        zt_ps = seq_ps(P, 32)
        nc.tensor.transpose(zt_ps[0:P, 0:32], z0s[:, 128 * t:128 * (t + 1)],
                            eye[0:32, 0:32])
        zt = pst.tile([P, 32], F32, tag=f"zT{t}", name=f"zT{t}")
        nc.scalar.copy(zt[:], zt_ps[0:P, 0:32])
        zT.append(zt)
        nc.vector.tensor_reduce(spack[:, t:t + 1], zt[:], axis=AXX, op=ALU.add)
        ja = pjk.tile([P, N], F32, tag="junka", name="junka")
        nc.scalar.activation(ja[:, 0:32], zt[:], ACTF.Square,
                             accum_out=qpack[:, t:t + 1])
    mu = pst.tile([P, 12], F32, tag="mu", name="mu")
    nc.vector.tensor_scalar(mu[:], spack[:], 1.0 / 32, 0.0, op0=ALU.mult)
    m2 = pst.tile([P, 12], F32, tag="m2", name="m2")
    nc.vector.tensor_tensor(m2[:], mu[:], mu[:], op=ALU.mult)
    var = pst.tile([P, 12], F32, tag="var", name="var")
    nc.vector.scalar_tensor_tensor(var[:], qpack[:], 1.0 / 32, m2[:],
                                   op0=ALU.mult, op1=ALU.subtract)
    eps_col = pst.tile([P, 1], F32, tag="eps_col", name="eps_col")
    nc.any.memset(eps_col[:], 1e-5)
    sd = pst.tile([P, 12], F32, tag="sd", name="sd")
    nc.scalar.activation(sd[:], var[:], ACTF.Sqrt, bias=eps_col[:])
    inv = pst.tile([P, 12], F32, tag="inv", name="inv")
    nc.vector.reciprocal(inv[:], sd[:])
    gam = wslice('bn_scale')
    bet = wslice('bn_shift')
    sc = pst.tile([P, 12], F32, tag="sc", name="sc")
    nc.vector.tensor_tensor(sc[:], inv[:], gam, op=ALU.mult)
    bi = pst.tile([P, 12], F32, tag="bi", name="bi")
    nc.vector.tensor_tensor(bi[:], mu[:], sc[:], op=ALU.mult)
    nc.vector.tensor_tensor(bi[:], bet, bi[:], op=ALU.subtract)
    zcur = []
    for t in range(12):
        zc = pst.tile([P, 32], F16, tag=f"zc{t}", name=f"zc{t}")
        nc.scalar.activation(zc[:], zT[t][:], ACTF.Identity,
                             bias=bi[:, t:t + 1], scale=sc[:, t:t + 1])
        zcur.append(zc[:])

    linBc = wslice('linBcol')
    for li in range(LIN_D):
        wt = wts[li]
        psA = seq_ps(P, 32)
        psB = seq_ps(64, 32)
        for k in range(12):
            wa = wt[:, HSH * k:HSH * k + 128]
            wb = wt[:, HSH * k + 128:HSH * (k + 1)]
            nc.tensor.matmul(psA[:], wa, zcur[k], start=(k == 0),
                             stop=(k == 11))
            nc.tensor.matmul(psB[:], wb, zcur[k], start=(k == 0),
                             stop=(k == 11))
        zslA = pwork.tile([P, 32], F16, tag="zslA", name="zslA")
        zslB = pwork.tile([64, 32], F16, tag="zslB", name="zslB")
        nc.scalar.activation(zslA[:], psA[:], ACTF.Lrelu,
                             bias=linBc[:, 2 * li:2 * li + 1], alpha=0.01)
        nc.scalar.activation(zslB[:], psB[0:64, :], ACTF.Lrelu,
                             bias=linBc[0:64, 2 * li + 1:2 * li + 2],
                             alpha=0.01)
        if li < LIN_D - 1:
            dma(io['zsl'][li][0:128, :], zslA[:])
            dma(io['zsl'][li][128:192, :], zslB[:])
            nc.gpsimd.collective_compute(
                "AllGather", ALU.bypass, replica_groups=[cores],
                ins=[io['zsl'][li][:]], outs=[io['zfull'][li][:]])
            zcat = pst.tile([P, 12 * 32], F16, tag=f"zcat{li}", name=f"zcat{li}")
            dma(zcat[:].rearrange("p (k b) -> p k b", k=12),
                io['zfull'][li].rearrange("(k p) b -> p k b", k=12))
            zcur = [zcat[:, 32 * t:32 * (t + 1)] for t in range(12)]
    # final layer: each core holds its 192-row slice of z4; compute the
    # local contribution to out = z4 @ out_W, AllGather the partials, sum.
    outWc = wslice('outWc')
    outWh = pwork.tile([P, 2], F16, tag="outWh", name="outWh")
    nc.vector.tensor_copy(outWh[:], outWc)
    outb_sb = wslice('outb', 1)
    ps_out = seq_ps(1, 32)
    nc.tensor.matmul(ps_out[:], outWh[:, 0:1], zslA[:],
                     start=True, stop=False)
    nc.tensor.matmul(ps_out[:], outWh[0:64, 1:2], zslB[:],
                     start=False, stop=True)
    o32 = pwork.tile([1, 32], F32, tag="o32", name="o32")
    nc.scalar.copy(o32[:], ps_out[:])
    dma(io['opart'][:], o32[:])
    nc.gpsimd.collective_compute(
        "AllGather", ALU.bypass, replica_groups=[cores],
        ins=[io['opart'][:]], outs=[io['opart8'][:]])
    op8 = pwork.tile([8, 32], F32, tag="op8", name="op8", padded_shape=[128, 32])
    dma(op8[:], io['opart8'][:])
    osum_ps = seq_ps(1, 32)
    nc.tensor.matmul(osum_ps[:], allones[0:8, 0:1], op8[0:8, :],
                     start=True, stop=True)
    o32f = pwork.tile([1, 32], F32, tag="o32f", name="o32f")
    nc.scalar.activation(o32f[:], osum_ps[:], ACTF.Identity, bias=outb_sb)
    dma(io['out32'][:], o32f[:])


def build_nc(skip_head=False):
    nc = bacc.Bacc("TRN2", target_bir_lowering=False, debug=False,
                   num_devices=NCORES)
    io = {}

    def inp(name, shape):
        io[name] = nc.dram_tensor(name, list(shape), F32, kind="ExternalInput").ap()

    inp('xpack', (GPC, 128, 2 * N + 24))
    inp('wpack', (128, WPACK_COLS))
    io['linW'] = nc.dram_tensor("linW", [LIN_D, 12, 128, HSH], F16,
                                kind="ExternalInput").ap()

    io['z0loc'] = nc.dram_tensor(
        "z0loc", [GPC, DIM2], F32,
        kind="ExternalOutput" if skip_head else "Internal").ap()
    io['out32'] = nc.dram_tensor("out32", [1, 32], F32,
                                 kind="ExternalOutput").ap()
    if not skip_head:
        io['z0all'] = nc.dram_tensor("z0all", [B, DIM2], F32,
                                     addr_space="Shared").ap()
        io['zsl'] = [nc.dram_tensor(f"zsl{li}", [HSH, 32], F16).ap()
                     for li in range(LIN_D)]
        io['zfull'] = [nc.dram_tensor(f"zfull{li}", [DIM2, 32], F16,
                                      addr_space="Shared").ap()
                       for li in range(LIN_D - 1)]
        io['opart'] = nc.dram_tensor("opart", [1, 32], F32).ap()
        io['opart8'] = nc.dram_tensor("opart8", [8, 32], F32,
                                      addr_space="Shared").ap()

    with tile.TileContext(nc) as tc:
        core_program(tc, io, skip_head=skip_head)
    nc.compile()
    return nc


_CACHED = {}
_DEBUG_IO = None


def kernel(**inputs) -> np.ndarray:
    from concourse.bass_utils import run_bass_kernel_spmd
    if 'nc' not in _CACHED:
        _CACHED['nc'] = build_nc()
    nc = _CACHED['nc']
    in_maps = []
    for c in range(NCORES):
        d = prep_host(inputs, c)
        in_maps.append({k: np.ascontiguousarray(v) for k, v in d.items()})
    res = run_bass_kernel_spmd(nc, in_maps, core_ids=list(range(NCORES)),
                               trace=bool(os.environ.get("KBENCH_TRACE")))
    _CACHED['last'] = res
    return res.results[0]['out32'].reshape(-1).astype(np.float32)


if __name__ == "__main__":
    data = dict(np.load('/root/problem/inputs.npz'))
    out = kernel(**data)
    print("kernel out:", out[:5])

